# revision 12
# baseline (speedup 1.0000x reference)
"""Bass/Tile TRN2 kernel for nn_DocLSTM (BiLSTM doc encoder + two-branch
block-sparse attention + feature projection), SPMD over 8 NeuronCores.

Structure:
  Launch A: embedding gather (dma_gather transpose mode, int16 two-pass
            split table) + batched BiLSTM over 2049 sequences sharded by
            sequence across the 8 cores -> hid [2049, 200].
  Host glue: similarity logits, sigmoid, softmax, attend matrix, mask,
            stable-partition orders (tiny O(S*D2) work).
  Launch B: per-branch multihead attention (softmax over the *query* axis,
            computed exactly via per-key normalizers Z_t) sharded by
            sentence stripe, plus the 327MB feat_w contraction sharded by
            the same stripes.  Two phases: phase 1 computes the Z_t
            partials (scores+exp only), an AllReduce combines them, and
            phase 2 recomputes scores and applies  out = A + (E-1) @ V/Z
            so the bf16 matmul never sees the near-constant part of E.
"""

import numpy as np
import ml_dtypes

bf = ml_dtypes.bfloat16

V, D, M, H, S, W, D2 = 50000, 300, 100, 8, 2048, 64, 200
NCORES = 8
BP = 264                  # padded sequences per core (max real = 257)
TOK = W * BP              # gather slots per core (16896, %128==0)
NCH = 8                   # X chunk tiles (8 steps each)
CHTOK = TOK // NCH        # 2112 tokens per X chunk
GSUB = 768                # idxs per dma_gather call (HW ring limit < 1024)
NGS = TOK // GSUB         # 22 gather calls per pass
EW = 384                  # padded embedding row (bf16 -> 768B, %256==0)
SPLIT = 32768             # int16 index limit for dma_gather
VROWS = V + 1             # + zero row for the pass-B redirect
GPERM = [0, 1, 3, 2]      # gate slot -> pytorch row block (i, f, o, g)

_cacheA = {}
_cacheB = {}


def _wrap_idx(ids):
    """Token list -> dma_gather index tile [128, n/16] int16.

    Position i lives at partition i%16, column i//16; the 16-partition block
    is replicated to all 128 partitions (one copy per GPSIMD core).
    """
    n = len(ids)
    out = np.zeros((16, n // 16), np.int16)
    out[np.arange(n) % 16, np.arange(n) // 16] = ids
    return np.tile(out, (8, 1))


def _core_seq_ranges():
    """Global sequence index ranges per core. Sequence 0 is the head (rsent)."""
    ranges = []
    start = 0
    for c in range(NCORES):
        nreal = 257 if c == 0 else 256
        ranges.append((start, nreal))
        start += nreal
    return ranges


# ---------------------------------------------------------------- launch A

def _build_A():
    from contextlib import ExitStack
    import concourse.bacc as bacc
    import concourse.tile as tile
    import concourse.mybir as mybir

    dt = mybir.dt
    AF = mybir.ActivationFunctionType
    nc = bacc.Bacc("TRN2", target_bir_lowering=False, debug=False,
                   num_devices=NCORES)
    etab = nc.dram_tensor("etab", [VROWS, EW], dt.bfloat16,
                          kind="ExternalInput").ap()
    idxa = nc.dram_tensor("idxa", [128, TOK // 16], dt.int16,
                          kind="ExternalInput").ap()
    idxb = nc.dram_tensor("idxb", [128, TOK // 16], dt.int16,
                          kind="ExternalInput").ap()
    wih = nc.dram_tensor("wih", [128, 24, M], dt.bfloat16,
                         kind="ExternalInput").ap()
    whh = nc.dram_tensor("whh", [M, 8, M], dt.bfloat16,
                         kind="ExternalInput").ap()
    hidT = nc.dram_tensor("hidT", [D2, BP], dt.bfloat16,
                          kind="ExternalOutput").ap()

    with tile.TileContext(nc) as tc, ExitStack() as ctx:
        pC = ctx.enter_context(tc.tile_pool(name="const", bufs=1))
        pX = ctx.enter_context(tc.tile_pool(name="xt", bufs=1))
        pG = ctx.enter_context(tc.tile_pool(name="gst", bufs=1))
        pSt = ctx.enter_context(tc.tile_pool(name="state", bufs=1))
        pA = ctx.enter_context(tc.tile_pool(name="act", bufs=2))
        pP = ctx.enter_context(tc.tile_pool(name="ps", bufs=1, space="PSUM"))

        idxa_t = pC.tile([128, TOK // 16], dt.int16)
        nc.sync.dma_start(idxa_t[:], idxa[:])
        idxb_t = pC.tile([128, TOK // 16], dt.int16)
        nc.sync.dma_start(idxb_t[:], idxb[:])
        wih_t = pC.tile([128, 24, M], dt.bfloat16)
        nc.sync.dma_start(wih_t[:], wih[:])
        whh_t = pC.tile([M, 8, M], dt.bfloat16)
        nc.sync.dma_start(whh_t[:], whh[:])

        X = [pX.tile([128, 3, CHTOK], dt.bfloat16, tag=f"x{c}",
                     name=f"xchunk{c}") for c in range(NCH)]
        for j in range(NGS):
            ga = pG.tile([128, 3, GSUB], dt.bfloat16, tag="ga")
            gb = pG.tile([128, 3, GSUB], dt.bfloat16, tag="gb")
            isl = slice(j * (GSUB // 16), (j + 1) * (GSUB // 16))
            nc.gpsimd.dma_gather(
                out_ap=ga[:], in_ap=etab[0:SPLIT, :], idxs_ap=idxa_t[:, isl],
                num_idxs=GSUB, num_idxs_reg=GSUB, elem_size=EW,
                transpose=True)
            nc.gpsimd.dma_gather(
                out_ap=gb[:], in_ap=etab[SPLIT:VROWS, :],
                idxs_ap=idxb_t[:, isl],
                num_idxs=GSUB, num_idxs_reg=GSUB, elem_size=EW,
                transpose=True)
            # add into the X chunk tiles this sub-range spans
            lo = j * GSUB
            while lo < (j + 1) * GSUB:
                c = lo // CHTOK
                hi = min((j + 1) * GSUB, (c + 1) * CHTOK)
                s0 = lo - j * GSUB
                nc.vector.tensor_add(
                    X[c][:, :, lo - c * CHTOK:hi - c * CHTOK],
                    ga[:, :, s0:s0 + hi - lo], gb[:, :, s0:s0 + hi - lo])
                lo = hi

        h_t, c_t = [], []
        for d in range(2):
            ht = pSt.tile([M, BP], dt.bfloat16, tag=f"h{d}")
            nc.vector.memset(ht[:], 0.0)
            h_t.append(ht)
            ct = pSt.tile([M, BP], dt.float32, tag=f"c{d}")
            nc.vector.memset(ct[:], 0.0)
            c_t.append(ct)

        for t in range(W):
            for d in range(2):
                tok = t if d == 0 else W - 1 - t
                ch, off = divmod(tok, W // NCH)
                off *= BP
                ps = pP.tile([128, 2048], dt.float32, tag=f"ps{d}")
                for s in range(4):
                    col = 512 * s
                    for kc in range(3):
                        nc.tensor.matmul(
                            ps[0:M, col:col + BP],
                            lhsT=wih_t[:, d * 12 + s * 3 + kc, :],
                            rhs=X[ch][:, kc, off:off + BP],
                            start=(kc == 0), stop=False)
                    nc.tensor.matmul(
                        ps[0:M, col:col + BP],
                        lhsT=whh_t[:, d * 4 + s, :], rhs=h_t[d][:],
                        start=False, stop=True)
                psv = ps[0:M, :].rearrange("p (g n) -> p g n", n=512)
                sigm = pA.tile([M, 3, BP], dt.float32, tag=f"sg{d}")
                nc.scalar.activation(sigm[:], psv[:, 0:3, 0:BP], AF.Sigmoid)
                tanhg = pA.tile([M, BP], dt.float32, tag=f"tg{d}")
                nc.scalar.activation(tanhg[:], psv[:, 3, 0:BP], AF.Tanh)
                t1 = pA.tile([M, BP], dt.float32, tag=f"t1{d}")
                nc.vector.tensor_mul(t1[:], sigm[:, 0, :], tanhg[:])
                nc.vector.tensor_mul(c_t[d][:], c_t[d][:], sigm[:, 1, :])
                nc.vector.tensor_add(c_t[d][:], c_t[d][:], t1[:])
                tanhc = pA.tile([M, BP], dt.float32, tag=f"tc{d}")
                nc.scalar.activation(tanhc[:], c_t[d][:], AF.Tanh)
                nc.vector.tensor_mul(h_t[d][:], sigm[:, 2, :], tanhc[:])

        nc.sync.dma_start(hidT[0:M, :], h_t[0][:])
        nc.sync.dma_start(hidT[M:D2, :], h_t[1][:])

    nc.compile()
    return nc


def _prep_A(inputs):
    emb = np.ascontiguousarray(inputs["emb"], dtype=np.float32)
    emb_pad = np.zeros((VROWS, EW), np.float32)
    emb_pad[:V, :D] = emb
    emb_pad[:, D] = 0.5       # bias lane: two gather passes sum to 1.0
    emb_pad[V, :D] = 0.0      # pass-B redirect row
    etab_np = emb_pad.astype(bf)

    tok_all = np.concatenate(
        [np.asarray(inputs["rsent"], np.int64)[None, :],
         np.asarray(inputs["body_sents"], np.int64)], axis=0)  # [2049, 64]

    idx_maps = []
    for c, (g0, nreal) in enumerate(_core_seq_ranges()):
        grid = np.zeros((W, BP), np.int64)
        grid[:, :nreal] = tok_all[g0:g0 + nreal].T  # [W, nreal]
        ids = grid.reshape(-1)
        ida = np.where(ids < SPLIT, ids, 0).astype(np.int16)
        idb = np.where(ids >= SPLIT, ids - SPLIT,
                       VROWS - 1 - SPLIT).astype(np.int16)
        idx_maps.append((_wrap_idx(ida), _wrap_idx(idb)))

    wih_np = np.zeros((2, 4, 3, 128, M), np.float32)
    whh_np = np.zeros((2, 4, M, M), np.float32)
    for d, (w_ih, w_hh, b_ih, b_hh) in enumerate([
            (inputs["w_ih_f"], inputs["w_hh_f"], inputs["b_ih_f"], inputs["b_hh_f"]),
            (inputs["w_ih_b"], inputs["w_hh_b"], inputs["b_ih_b"], inputs["b_hh_b"])]):
        btot = (np.asarray(b_ih, np.float32) + np.asarray(b_hh, np.float32))
        wT = np.zeros((EW, 4 * M), np.float32)
        wT[:D, :] = np.asarray(w_ih, np.float32).T
        wT[D, :] = btot
        for s, blk in enumerate(GPERM):
            for kc in range(3):
                wih_np[d, s, kc] = wT[128 * kc:128 * (kc + 1),
                                      M * blk:M * (blk + 1)]
            whh_np[d, s] = np.asarray(w_hh, np.float32).T[:, M * blk:M * (blk + 1)]
    wih_sb = np.ascontiguousarray(
        wih_np.transpose(3, 0, 1, 2, 4).reshape(128, 24, M)).astype(bf)
    whh_sb = np.ascontiguousarray(
        whh_np.transpose(2, 0, 1, 3).reshape(M, 8, M)).astype(bf)

    in_maps = []
    for c in range(NCORES):
        in_maps.append({"etab": etab_np, "idxa": idx_maps[c][0],
                        "idxb": idx_maps[c][1], "wih": wih_sb,
                        "whh": whh_sb})
    return in_maps


def _run_A(inputs):
    from concourse.bass_utils import run_bass_kernel_spmd
    if "nc" not in _cacheA:
        _cacheA["nc"] = _build_A()
    nc = _cacheA["nc"]
    in_maps = _prep_A(inputs)
    res = run_bass_kernel_spmd(nc, in_maps, list(range(NCORES)))
    hid = np.zeros((S + 1, D2), np.float32)
    for c, (g0, nreal) in enumerate(_core_seq_ranges()):
        hT = res.results[c]["hidT"].view(bf).astype(np.float32)  # [200, BP]
        hid[g0:g0 + nreal] = hT[:, :nreal].T
    return hid


# ---------------------------------------------------------------- launch B

def _build_B(key):
    """key = (n_pad, sp, nbr, signs, aligned) — branch-structure parameters."""
    n_pad, sp, nbr, signs, aligned = key
    from contextlib import ExitStack
    import concourse.bacc as bacc
    import concourse.tile as tile
    import concourse.mybir as mybir
    from concourse.masks import make_identity

    dt = mybir.dt
    AF = mybir.ActivationFunctionType
    KCH = sp // 128               # s-tiles per stripe
    TT = n_pad // 128             # key tiles
    assert TT * 128 == n_pad and KCH * 128 == sp and sp <= 512
    assert H * TT <= 128, "Z layout requires H*TT <= 128"
    NROW = sp * D2                # feat rows per stripe
    NSC = (NROW + 2047) // 2048   # feat super-chunks
    scale = 1.0 / float(np.sqrt(np.float32(D2)))

    nc = bacc.Bacc("TRN2", target_bir_lowering=False, debug=False,
                   num_devices=NCORES)
    xf = [nc.dram_tensor(f"xf{b}", [128, 2, n_pad], dt.bfloat16,
                         kind="ExternalInput").ap() for b in range(nbr)]
    xq = [nc.dram_tensor(f"xq{b}", [128, 2, sp], dt.bfloat16,
                         kind="ExternalInput").ap() for b in range(nbr)]
    qkv = nc.dram_tensor("qkv", [128, H, 2, 3, D2], dt.bfloat16,
                         kind="ExternalInput").ap()
    cwb = nc.dram_tensor("cwb", [128, 16, D2], dt.bfloat16,
                         kind="ExternalInput").ap()
    cwf = nc.dram_tensor("cwf", [128, 16, D2], dt.float32,
                         kind="ExternalInput").ap()
    cbias = nc.dram_tensor("cbias", [M, 2], dt.float32,
                           kind="ExternalInput").ap()
    vmask = nc.dram_tensor("vmask", [128, nbr, 128], dt.float32,
                           kind="ExternalInput").ap()
    vbias = nc.dram_tensor("vbias", [128, H, D2], dt.float32,
                           kind="ExternalInput").ap()
    smask = nc.dram_tensor("smask", [1, nbr, sp], dt.float32,
                           kind="ExternalInput").ap()
    ftd = nc.dram_tensor("ftd", [NSC, 128, 16, D2], dt.float32,
                         kind="ExternalInput").ap()
    fpart = nc.dram_tensor("fpart", [nbr, D2], dt.float32,
                           kind="ExternalOutput").ap()

    def head_scores(pP, pT, nc, xf_t, xq_t, k_sb, q_sb, qkv_t, h, b, emit_e):
        """Compute scores^T tiles for head h; yields (tt, psum_tile)."""
        for m in range(2):
            for tchunk in range(n_pad // 512):
                psk = pP.tile([128, 512], dt.float32, tag="pbig", name="psk")
                for kc in range(2):
                    nc.tensor.matmul(
                        psk[0:M, :],
                        lhsT=qkv_t[:, h, kc, 1, M * m:M * (m + 1)],
                        rhs=xf_t[:, kc, 512 * tchunk:512 * (tchunk + 1)],
                        start=(kc == 0), stop=(kc == 1))
                nc.scalar.copy(
                    k_sb[:, m, 512 * tchunk:512 * (tchunk + 1)], psk[0:M, :])
        for m in range(2):
            psq = pP.tile([128, 512], dt.float32, tag="pbig", name="psq")
            for kc in range(2):
                nc.tensor.matmul(
                    psq[0:M, 0:sp], lhsT=qkv_t[:, h, kc, 0, M * m:M * (m + 1)],
                    rhs=xq_t[:, kc, :], start=(kc == 0), stop=(kc == 1))
            nc.scalar.copy(q_sb[:, m, :], psq[0:M, 0:sp])
        for tt in range(TT):
            pss = pP.tile([128, 512], dt.float32, tag="pbig", name="pss")
            for m in range(2):
                nc.tensor.matmul(
                    pss[:, 0:sp], lhsT=k_sb[:, m, 128 * tt:128 * (tt + 1)],
                    rhs=q_sb[:, m, :], start=(m == 0), stop=(m == 1))
            yield tt, pss

    with tile.TileContext(nc) as tc, ExitStack() as ctx:
        pC = ctx.enter_context(tc.tile_pool(name="const", bufs=1))
        pB = ctx.enter_context(tc.tile_pool(name="big", bufs=1))
        pT = ctx.enter_context(tc.tile_pool(name="tmp", bufs=2))
        pF = ctx.enter_context(tc.tile_pool(name="ftst", bufs=3))
        pP = ctx.enter_context(tc.tile_pool(name="ps", bufs=2, space="PSUM"))
        pPacc = ctx.enter_context(tc.tile_pool(name="psa", bufs=1,
                                               space="PSUM"))
        pD = ctx.enter_context(tc.tile_pool(name="dram", bufs=1, space="DRAM"))

        cw_b = pC.tile([128, 16, D2], dt.bfloat16)
        nc.sync.dma_start(cw_b[:], cwb[:])
        cw_f = pC.tile([128, 16, D2], dt.float32)
        nc.sync.dma_start(cw_f[:], cwf[:])
        cb_t = pC.tile([M, 2], dt.float32)
        nc.sync.dma_start(cb_t[:], cbias[:])
        vm_t = pC.tile([128, nbr, 128], dt.float32)
        nc.sync.dma_start(vm_t[:], vmask[:])
        vb_t = pC.tile([128, H, D2], dt.float32)
        nc.sync.dma_start(vb_t[:], vbias[:])
        ones_t = pC.tile([128, 1], dt.bfloat16)
        nc.vector.memset(ones_t[:], 1.0)
        sm_t = None
        if not aligned:
            sm_t = pC.tile([1, nbr, sp], dt.float32)
            nc.sync.dma_start(sm_t[:], smask[:])
        qkv_t = pC.tile([128, H, 2, 3, D2], dt.bfloat16)
        nc.sync.dma_start(qkv_t[:], qkv[:])
        idn = pC.tile([128, 128], dt.float32)
        make_identity(nc, idn[:])

        mht_d = pD.tile([NSC * 2048, nbr], dt.float32)
        zin_d = pD.tile([128, nbr * 128], dt.float32)
        zout_d = pD.tile([128, nbr * 128], dt.float32)

        fps = pPacc.tile([nbr, D2], dt.float32)  # feat accumulator (pinned)

        xf_ts, xq_ts = [], []
        for b in range(nbr):
            xf_t = pB.tile([128, 2, n_pad], dt.bfloat16, tag=f"xf{b}")
            nc.sync.dma_start(xf_t[:], xf[b][:])
            xf_ts.append(xf_t)
            xq_t = pB.tile([128, 2, sp], dt.bfloat16, tag=f"xq{b}")
            nc.sync.dma_start(xq_t[:], xq[b][:])
            xq_ts.append(xq_t)

        # ---------------- phase 1: Z partials
        for b in range(nbr):
            zp_sb = pB.tile([128, 128], dt.float32, tag=f"z{b}")
            nc.vector.memset(zp_sb[:], 0.125)  # unwritten cols -> Z=1 total
            k_sb = pT.tile([M, 2, n_pad], dt.bfloat16, tag="k1")
            q_sb = pT.tile([M, 2, sp], dt.bfloat16, tag="q1")
            for h in range(H):
                for tt, pss in head_scores(pP, pT, nc, xf_ts[b], xq_ts[b],
                                           k_sb, q_sb, qkv_t, h, b, False):
                    zcol = zp_sb[:, h * TT + tt:h * TT + tt + 1]
                    e_t = pT.tile([128, sp], dt.float32, tag="e1")
                    if aligned:
                        nc.scalar.activation(
                            e_t[:], pss[:, 0:sp], AF.Exp,
                            scale=float(signs[b]) * scale, accum_out=zcol)
                    else:
                        nc.scalar.activation(
                            e_t[:], pss[:, 0:sp], AF.Exp,
                            scale=float(signs[b]) * scale)
                        ez = pT.tile([128, sp], dt.float32, tag="ez1")
                        nc.vector.tensor_mul(
                            ez[:], e_t[:],
                            sm_t[0:1, b, :].to_broadcast([128, sp]))
                        nc.vector.reduce_sum(zcol, ez[:],
                                             axis=mybir.AxisListType.X)
            nc.gpsimd.dma_start(zin_d[:, 128 * b:128 * (b + 1)], zp_sb[:])

        nc.gpsimd.collective_compute(
            "AllReduce", mybir.AluOpType.add,
            replica_groups=[list(range(NCORES))],
            ins=[zin_d.opt()], outs=[zout_d.opt()])

        # ---------------- phase 2: apply
        for b in range(nbr):
            zall = pT.tile([128, 128], dt.float32, tag="zall")
            nc.gpsimd.dma_start(zall[:], zout_d[:, 128 * b:128 * (b + 1)])
            zr = pB.tile([128, 128], dt.float32, tag=f"zr{b}")
            nc.vector.reciprocal(zr[:], zall[:])
            nc.vector.tensor_mul(zr[:], zr[:], vm_t[:, b, :])

            out_sb = pB.tile([128, 16, sp], dt.bfloat16, tag=f"o{b}")
            a_sb = pB.tile([128, 16, 1], dt.float32, tag=f"a{b}")
            nc.vector.memset(a_sb[:], 0.0)
            k_sb = pT.tile([M, 2, n_pad], dt.bfloat16, tag="k1")
            q_sb = pT.tile([M, 2, sp], dt.bfloat16, tag="q1")

            for h in range(H):
                em_sb = pT.tile([128, TT, sp], dt.bfloat16, tag="em")
                for tt, pss in head_scores(pP, pT, nc, xf_ts[b], xq_ts[b],
                                           k_sb, q_sb, qkv_t, h, b, True):
                    e_t = pT.tile([128, sp], dt.float32, tag="e2")
                    nc.scalar.activation(e_t[:], pss[:, 0:sp], AF.Exp,
                                         scale=float(signs[b]) * scale)
                    nc.vector.tensor_scalar_add(em_sb[:, tt, :], e_t[:], -1.0)
                vp_sb = pT.tile([128, TT, 256], dt.bfloat16, tag="vp")
                nc.vector.memset(vp_sb[:], 0.0)
                psa0 = pP.tile([128, 1], dt.float32, tag="psmall", name="psa0")
                psa1 = pP.tile([128, 1], dt.float32, tag="psmall", name="psa1")
                for tt in range(TT):
                    psv = pP.tile([128, 512], dt.float32, tag="pbig", name="psv")
                    for kc in range(2):
                        nc.tensor.matmul(
                            psv[:, 0:D2],
                            lhsT=xf_ts[b][:, kc, 128 * tt:128 * (tt + 1)],
                            rhs=qkv_t[:, h, kc, 2, :],
                            start=(kc == 0), stop=(kc == 1))
                    vtmp = pT.tile([128, D2], dt.float32, tag="vtmp")
                    nc.vector.tensor_tensor(
                        out=vtmp[:], in0=psv[:, 0:D2],
                        in1=vb_t[:, h, :],
                        op=mybir.AluOpType.add)
                    nc.vector.tensor_scalar_mul(
                        vp_sb[:, tt, 0:D2], vtmp[:],
                        zr[:, h * TT + tt:h * TT + tt + 1])
                    nc.tensor.matmul(
                        psa0[:, :], lhsT=vtmp[:, 0:128],
                        rhs=zr[:, h * TT + tt:h * TT + tt + 1],
                        start=(tt == 0), stop=(tt == TT - 1))
                    nc.tensor.matmul(
                        psa1[0:D2 - 128, :], lhsT=vtmp[:, 128:D2],
                        rhs=zr[:, h * TT + tt:h * TT + tt + 1],
                        start=(tt == 0), stop=(tt == TT - 1))
                nc.scalar.copy(a_sb[0:128, 2 * h, :], psa0[:, :])
                nc.scalar.copy(a_sb[0:D2 - 128, 2 * h + 1, :],
                               psa1[0:D2 - 128, :])
                for j in range(2):
                    pso = pP.tile([128, 512], dt.float32, tag="pbig", name="pso")
                    for tt in range(TT):
                        nc.tensor.matmul(
                            pso[:, 0:sp],
                            lhsT=vp_sb[:, tt, 128 * j:128 * (j + 1)],
                            rhs=em_sb[:, tt, :],
                            start=(tt == 0), stop=(tt == TT - 1))
                    nc.scalar.copy(out_sb[:, 2 * h + j, :], pso[:, 0:sp])

            # mh^T = cw^T @ out (+ cw^T @ A + concat_b as f32 bias)
            mh_sb = pT.tile([M, 2, sp], dt.float32, tag="mh")
            for bc in range(2):
                psm1 = pP.tile([128, 1], dt.float32, tag="psmall", name="psm1")
                for k in range(16):
                    nc.tensor.matmul(
                        psm1[0:M, :], lhsT=cw_f[:, k, M * bc:M * (bc + 1)],
                        rhs=a_sb[:, k, :], start=(k == 0), stop=(k == 15))
                mha = pT.tile([M, 1], dt.float32, tag="mha")
                nc.vector.tensor_tensor(
                    out=mha[:], in0=psm1[0:M, :], in1=cb_t[:, bc:bc + 1],
                    op=mybir.AluOpType.add)
                psm = pP.tile([128, 512], dt.float32, tag="pbig", name="psm")
                for k in range(16):
                    nc.tensor.matmul(
                        psm[0:M, 0:sp], lhsT=cw_b[:, k, M * bc:M * (bc + 1)],
                        rhs=out_sb[:, k, :], start=(k == 0), stop=(k == 15))
                nc.scalar.activation(mh_sb[:, bc, :], psm[0:M, 0:sp],
                                     AF.Identity, bias=mha[:])

            # transpose mh^T -> token-major, store to MHT dram column b
            mtk = pT.tile([128, KCH, D2], dt.float32, tag="mtk")
            for bc in range(2):
                for k in range(KCH):
                    pst = pP.tile([128, 512], dt.float32, tag="pbig", name="pst")
                    nc.tensor.transpose(
                        pst[:, 0:M], mh_sb[:, bc, 128 * k:128 * (k + 1)],
                        idn[0:M, 0:M])
                    nc.scalar.copy(mtk[:, k, M * bc:M * (bc + 1)], pst[:, 0:M])
            mview = mht_d[0:KCH * 128 * D2, :].rearrange(
                "(k p c) r -> p k c r", p=128, c=D2)
            nc.sync.dma_start(mview[:, :, :, b], mtk[:])

        if NSC * 2048 > NROW:
            ztail = pT.tile([128, nbr], dt.float32, tag="ztail")
            nc.vector.memset(ztail[:], 0.0)
            tail = mht_d[:].rearrange("(c u p) r -> p c u r", c=NSC, u=16)
            for u in range(16):
                row0 = (NSC - 1) * 2048 + 128 * u
                if row0 >= NROW:
                    nc.sync.dma_start(tail[:, NSC - 1, u, :], ztail[:])

        # feat contraction: fpart += MHT_chunk.T @ featT_chunk
        for c in range(NSC):
            ft_t = pF.tile([128, 16, D2], dt.float32, tag="ft")
            nc.sync.dma_start(ft_t[:], ftd[c])
            mh_t = pF.tile([128, 16, nbr], dt.float32, tag="mht")
            nc.sync.dma_start(
                mh_t[:],
                mht_d[:].rearrange("(c u p) r -> p c u r",
                                   c=NSC, u=16)[:, c, :, :])
            for u in range(16):
                nc.tensor.matmul(
                    fps[:, :], lhsT=mh_t[:, u, :], rhs=ft_t[:, u, :],
                    start=(c == 0 and u == 0),
                    stop=(c == NSC - 1 and u == 15))
        ot = pT.tile([nbr, D2], dt.float32, tag="ot")
        nc.scalar.copy(ot[:], fps[:])
        nc.sync.dma_start(fpart[:], ot[:])

    nc.compile()
    return nc


def _prep_B(inputs, branches, n_pad, sp, aligned):
    """branches: list of (X_sorted [S, D2] f32, n_valid, sign)."""
    nbr = len(branches)
    TT = n_pad // 128
    qkv_np = np.zeros((2, 3, H, 128, D2), np.float32)
    for w_i, wkey, bkey in ((0, "qw", "qb"), (1, "kw", "kb"), (2, "vw", "vb")):
        wa = np.asarray(inputs[wkey], np.float32)   # [H, 200, 200] (out, in)
        ba = np.asarray(inputs[bkey], np.float32)   # [H, 200]
        for h in range(H):
            wT = np.zeros((256, D2), np.float32)
            wT[:D2] = wa[h].T
            if w_i != 2:
                wT[D2] = ba[h]   # v bias added in f32 on device instead
            qkv_np[0, w_i, h] = wT[:128]
            qkv_np[1, w_i, h] = wT[128:]
    vbias_np = np.ascontiguousarray(np.broadcast_to(
        np.asarray(inputs["vb"], np.float32)[None, :, :], (128, H, D2)))
    qkv_sb = np.ascontiguousarray(
        qkv_np.transpose(3, 2, 0, 1, 4)).astype(bf)  # [128, H, 2, 3, 200]

    cw = np.asarray(inputs["concat_w"], np.float32)  # [200, 1600]
    cwp = np.zeros((2048, D2), np.float32)
    for h in range(H):
        cwp[256 * h:256 * h + D2] = cw[:, D2 * h:D2 * (h + 1)].T
    cw_sb = np.ascontiguousarray(cwp.reshape(16, 128, D2).transpose(1, 0, 2))
    cbias_np = np.ascontiguousarray(
        np.asarray(inputs["concat_b"], np.float32).reshape(2, M).T)

    xf_list = []
    vm = np.zeros((128, nbr, 128), np.float32)
    for b, (Xs, n, sign) in enumerate(branches):
        xfp = np.zeros((256, n_pad), np.float32)
        xfp[:D2, :n] = Xs[:n].T
        xfp[D2, :n] = 1.0
        xf_list.append(np.ascontiguousarray(
            xfp.reshape(2, 128, n_pad).transpose(1, 0, 2)).astype(bf))
        for tt in range(TT):
            valid = (np.arange(128) + 128 * tt) < n
            for h in range(H):
                vm[:, b, h * TT + tt] = valid

    featT = np.ascontiguousarray(np.asarray(inputs["feat_w"], np.float32).T)
    NROW = sp * D2
    NSC = (NROW + 2047) // 2048

    in_maps = []
    for c in range(NCORES):
        s0 = c * sp
        m = {"qkv": qkv_sb, "cwb": cw_sb.astype(bf), "cwf": cw_sb,
             "cbias": cbias_np, "vmask": vm, "vbias": vbias_np}
        sm = np.zeros((1, nbr, sp), np.float32)
        for b, (Xs, n, sign) in enumerate(branches):
            xq = np.zeros((256, sp), np.float32)
            valid_cols = max(0, min(sp, n - s0))
            if valid_cols > 0:
                xq[:D2, :valid_cols] = Xs[s0:s0 + valid_cols].T
                xq[D2, :valid_cols] = 1.0
                sm[0, b, :valid_cols] = 1.0
            m[f"xf{b}"] = xf_list[b]
            m[f"xq{b}"] = np.ascontiguousarray(
                xq.reshape(2, 128, sp).transpose(1, 0, 2)).astype(bf)
        m["smask"] = sm
        stripe = np.zeros((NSC * 2048, D2), np.float32)
        r0 = s0 * D2
        rows = max(0, min(NROW, featT.shape[0] - r0))
        if rows > 0:
            stripe[:rows] = featT[r0:r0 + rows]
        m["ftd"] = np.ascontiguousarray(
            stripe.reshape(NSC, 16, 128, D2).transpose(0, 2, 1, 3))
        in_maps.append(m)
    return in_maps


def _run_B(inputs, branches):
    from concourse.bass_utils import run_bass_kernel_spmd
    nmax = max(n for _, n, _ in branches)
    sp = -(-nmax // (NCORES * 128)) * 128
    n_pad = sp * NCORES
    aligned = all(n == n_pad for _, n, _ in branches)
    signs = tuple(sign for _, _, sign in branches)
    key = (n_pad, sp, len(branches), signs, aligned)
    if key not in _cacheB:
        _cacheB[key] = _build_B(key)
    nc = _cacheB[key]
    in_maps = _prep_B(inputs, branches, n_pad, sp, aligned)
    res = run_bass_kernel_spmd(nc, in_maps, list(range(NCORES)))
    parts = np.stack([res.results[c]["fpart"] for c in range(NCORES)])
    return parts.sum(axis=0)  # [nbr, 200]


# ------------------------------------------------------------------ driver

def kernel(**inputs):
    hid = _run_A(inputs)

    head, sents = hid[0], hid[1:]
    u = head.astype(np.float32) @ np.asarray(inputs["sim_w"], np.float32)
    logits = sents @ u + np.asarray(inputs["sim_b"], np.float32)[0]
    sig = (1.0 / (1.0 + np.exp(-logits))).astype(np.float32)
    e = np.exp(sig - sig.max())
    prob = (e / e.sum()).astype(np.float32)
    attend = (prob[:, None] * sents).astype(np.float32)

    mask = sig >= 0.5
    n_high = int(mask.sum())
    n_low = S - n_high
    feat_b = np.asarray(inputs["feat_b"], np.float32)

    branches = []
    slot = {}
    if n_high > 0:
        order = np.argsort(np.where(mask, 0, 1), kind="stable")
        slot[0] = len(branches)
        branches.append((attend[order], n_high, 1.0))
    if n_low > 0:
        order = np.argsort(np.where(mask, 1, 0), kind="stable")
        slot[1] = len(branches)
        branches.append((attend[order], n_low, -1.0))

    out = np.zeros((2, D2), np.float32)
    if branches:
        parts = _run_B(inputs, branches)
        for r in range(2):
            out[r] = (parts[slot[r]] + feat_b) if r in slot else feat_b
    else:
        out[0] = feat_b
        out[1] = feat_b
    return out.astype(np.float32)


# revision 36
# speedup vs baseline: 1.5531x; 1.5531x over previous
"""Bass/Tile TRN2 kernel for nn_DocLSTM (BiLSTM doc encoder + two-branch
block-sparse attention + feature projection), SPMD over 8 NeuronCores.

Structure:
  Launch A: embedding gather (dma_gather transpose mode, int16 two-pass
            split table) + batched BiLSTM over 2049 sequences sharded by
            sequence across the 8 cores -> hid [2049, 200].
  Host glue: similarity logits, sigmoid, softmax, attend matrix, mask,
            stable-partition orders (tiny O(S*D2) work).
  Launch B: per-branch multihead attention (softmax over the *query* axis,
            computed exactly via per-key normalizers Z_t) sharded by
            sentence stripe, plus the 327MB feat_w contraction sharded by
            the same stripes.  Two phases: phase 1 computes the Z_t
            partials (scores+exp only), an AllReduce combines them, and
            phase 2 recomputes scores and applies  out = A + (E-1) @ V/Z
            so the bf16 matmul never sees the near-constant part of E.
"""

import numpy as np
import ml_dtypes

bf = ml_dtypes.bfloat16

V, D, M, H, S, W, D2 = 50000, 300, 100, 8, 2048, 64, 200
NCORES = 8
BP = 264                  # padded sequences per core (max real = 257)
TOK = W * BP              # gather slots per core (16896, %128==0)
NCH = 8                   # X chunk tiles (8 steps each)
CHTOK = TOK // NCH        # 2112 tokens per X chunk
GSUB = 768                # idxs per dma_gather call (HW ring limit < 1024)
NGS = TOK // GSUB         # 22 gather calls per pass
EW = 384                  # padded embedding row (bf16 -> 768B, %256==0)
SPLIT = 32768             # int16 index limit for dma_gather
VROWS = V + 1             # + zero row for the pass-B redirect
GPERM = [0, 1, 3, 2]      # gate slot -> pytorch row block (i, f, o, g)

_cacheA = {}
_cacheB = {}


def _wrap_idx(ids):
    """Token list -> dma_gather index tile [128, n/16] int16.

    Position i lives at partition i%16, column i//16; the 16-partition block
    is replicated to all 128 partitions (one copy per GPSIMD core).
    """
    n = len(ids)
    out = np.zeros((16, n // 16), np.int16)
    out[np.arange(n) % 16, np.arange(n) // 16] = ids
    return np.tile(out, (8, 1))


def _core_seq_ranges():
    """Global sequence index ranges per core. Sequence 0 is the head (rsent)."""
    ranges = []
    start = 0
    for c in range(NCORES):
        nreal = 257 if c == 0 else 256
        ranges.append((start, nreal))
        start += nreal
    return ranges


# ---------------------------------------------------------------- launch A

def _build_A():
    from contextlib import ExitStack
    import concourse.bacc as bacc
    import concourse.tile as tile
    import concourse.mybir as mybir

    dt = mybir.dt
    AF = mybir.ActivationFunctionType
    nc = bacc.Bacc("TRN2", target_bir_lowering=False, debug=False,
                   num_devices=NCORES)
    etab = nc.dram_tensor("etab", [VROWS, EW], dt.bfloat16,
                          kind="ExternalInput").ap()
    idxa = nc.dram_tensor("idxa", [128, TOK // 16], dt.int16,
                          kind="ExternalInput").ap()
    idxb = nc.dram_tensor("idxb", [128, TOK // 16], dt.int16,
                          kind="ExternalInput").ap()
    wih = nc.dram_tensor("wih", [128, 24, M], dt.bfloat16,
                         kind="ExternalInput").ap()
    whh = nc.dram_tensor("whh", [M, 8, M], dt.bfloat16,
                         kind="ExternalInput").ap()
    hidT = nc.dram_tensor("hidT", [D2, BP], dt.bfloat16,
                          kind="ExternalOutput").ap()

    with tile.TileContext(nc) as tc, ExitStack() as ctx:
        pC = ctx.enter_context(tc.tile_pool(name="const", bufs=1))
        pX = ctx.enter_context(tc.tile_pool(name="xt", bufs=1))
        pG = ctx.enter_context(tc.tile_pool(name="gst", bufs=2))
        pSt = ctx.enter_context(tc.tile_pool(name="state", bufs=1))
        pA = ctx.enter_context(tc.tile_pool(name="act", bufs=2))
        pP = ctx.enter_context(tc.tile_pool(name="ps", bufs=1, space="PSUM"))

        idxa_t = pC.tile([128, TOK // 16], dt.int16)
        nc.sync.dma_start(idxa_t[:], idxa[:])
        idxb_t = pC.tile([128, TOK // 16], dt.int16)
        nc.sync.dma_start(idxb_t[:], idxb[:])
        wih_t = pC.tile([128, 24, M], dt.bfloat16)
        nc.sync.dma_start(wih_t[:], wih[:])
        whh_t = pC.tile([M, 8, M], dt.bfloat16)
        nc.sync.dma_start(whh_t[:], whh[:])

        X = [pX.tile([128, 3, CHTOK], dt.bfloat16, tag=f"x{c}",
                     name=f"xchunk{c}") for c in range(NCH)]
        jorder = []
        for i in range((NGS + 1) // 2):
            jorder.append(i)
            if NGS - 1 - i != i:
                jorder.append(NGS - 1 - i)
        for j in jorder:
            ga = pG.tile([128, 3, GSUB], dt.bfloat16, tag="ga")
            gb = pG.tile([128, 3, GSUB], dt.bfloat16, tag="gb")
            isl = slice(j * (GSUB // 16), (j + 1) * (GSUB // 16))
            nc.gpsimd.dma_gather(
                out_ap=ga[:], in_ap=etab[0:SPLIT, :], idxs_ap=idxa_t[:, isl],
                num_idxs=GSUB, num_idxs_reg=GSUB, elem_size=EW,
                transpose=True)
            nc.gpsimd.dma_gather(
                out_ap=gb[:], in_ap=etab[SPLIT:VROWS, :],
                idxs_ap=idxb_t[:, isl],
                num_idxs=GSUB, num_idxs_reg=GSUB, elem_size=EW,
                transpose=True)
            # add into the X chunk tiles this sub-range spans
            lo = j * GSUB
            while lo < (j + 1) * GSUB:
                c = lo // CHTOK
                hi = min((j + 1) * GSUB, (c + 1) * CHTOK)
                s0 = lo - j * GSUB
                nc.vector.tensor_add(
                    X[c][:, :, lo - c * CHTOK:hi - c * CHTOK],
                    ga[:, :, s0:s0 + hi - lo], gb[:, :, s0:s0 + hi - lo])
                lo = hi

        h_t, c_t = [], []
        for d in range(2):
            ht = pSt.tile([M, BP], dt.bfloat16, tag=f"h{d}", name=f"h{d}")
            nc.vector.memset(ht[:], 0.0)
            h_t.append(ht)
            ct = pSt.tile([M, BP], dt.float32, tag=f"c{d}", name=f"c{d}")
            nc.vector.memset(ct[:], 0.0)
            c_t.append(ct)

        for t in range(W):
            for d in range(2):
                tok = t if d == 0 else W - 1 - t
                ch, off = divmod(tok, W // NCH)
                off *= BP
                # per-gate psum tiles; order g,i,f,o so the c-chain
                # (t1 = sig_i*tanh_g) can start as early as possible
                gact = {}
                for s in (2, 0, 1, 3):
                    gp = pP.tile([M, BP], dt.float32, tag=f"ps{d}g{s}",
                                 name=f"gp{d}{s}")
                    for kc in range(3):
                        nc.tensor.matmul(
                            gp[:, :],
                            lhsT=wih_t[:, d * 12 + s * 3 + kc, :],
                            rhs=X[ch][:, kc, off:off + BP],
                            start=(kc == 0), stop=False)
                    nc.tensor.matmul(
                        gp[:, :],
                        lhsT=whh_t[:, d * 4 + s, :], rhs=h_t[d][:],
                        start=False, stop=True)
                    av = pA.tile([M, BP], dt.float32, tag=f"ac{d}{s}",
                                 name=f"av{d}{s}")
                    nc.scalar.activation(
                        av[:], gp[:, :],
                        AF.Tanh if s == 2 else AF.Sigmoid)
                    gact[s] = av
                t1 = pA.tile([M, BP], dt.float32, tag=f"t1{d}")
                nc.vector.tensor_mul(t1[:], gact[0][:], gact[2][:])
                nc.vector.tensor_mul(c_t[d][:], c_t[d][:], gact[1][:])
                nc.vector.tensor_add(c_t[d][:], c_t[d][:], t1[:])
                tanhc = pA.tile([M, BP], dt.float32, tag=f"tc{d}")
                nc.scalar.activation(tanhc[:], c_t[d][:], AF.Tanh)
                nc.vector.tensor_mul(h_t[d][:], gact[3][:], tanhc[:])

        nc.sync.dma_start(hidT[0:M, :], h_t[0][:])
        nc.sync.dma_start(hidT[M:D2, :], h_t[1][:])

    nc.compile()
    return nc


def _prep_A(inputs):
    emb = np.ascontiguousarray(inputs["emb"], dtype=np.float32)
    emb_pad = np.zeros((VROWS, EW), np.float32)
    emb_pad[:V, :D] = emb
    emb_pad[:, D] = 0.5       # bias lane: two gather passes sum to 1.0
    emb_pad[V, :D] = 0.0      # pass-B redirect row
    etab_np = emb_pad.astype(bf)

    tok_all = np.concatenate(
        [np.asarray(inputs["rsent"], np.int64)[None, :],
         np.asarray(inputs["body_sents"], np.int64)], axis=0)  # [2049, 64]

    idx_maps = []
    for c, (g0, nreal) in enumerate(_core_seq_ranges()):
        grid = np.zeros((W, BP), np.int64)
        grid[:, :nreal] = tok_all[g0:g0 + nreal].T  # [W, nreal]
        ids = grid.reshape(-1)
        ida = np.where(ids < SPLIT, ids, 0).astype(np.int16)
        idb = np.where(ids >= SPLIT, ids - SPLIT,
                       VROWS - 1 - SPLIT).astype(np.int16)
        idx_maps.append((_wrap_idx(ida), _wrap_idx(idb)))

    wih_np = np.zeros((2, 4, 3, 128, M), np.float32)
    whh_np = np.zeros((2, 4, M, M), np.float32)
    for d, (w_ih, w_hh, b_ih, b_hh) in enumerate([
            (inputs["w_ih_f"], inputs["w_hh_f"], inputs["b_ih_f"], inputs["b_hh_f"]),
            (inputs["w_ih_b"], inputs["w_hh_b"], inputs["b_ih_b"], inputs["b_hh_b"])]):
        btot = (np.asarray(b_ih, np.float32) + np.asarray(b_hh, np.float32))
        wT = np.zeros((EW, 4 * M), np.float32)
        wT[:D, :] = np.asarray(w_ih, np.float32).T
        wT[D, :] = btot
        for s, blk in enumerate(GPERM):
            for kc in range(3):
                wih_np[d, s, kc] = wT[128 * kc:128 * (kc + 1),
                                      M * blk:M * (blk + 1)]
            whh_np[d, s] = np.asarray(w_hh, np.float32).T[:, M * blk:M * (blk + 1)]
    wih_sb = np.ascontiguousarray(
        wih_np.transpose(3, 0, 1, 2, 4).reshape(128, 24, M)).astype(bf)
    whh_sb = np.ascontiguousarray(
        whh_np.transpose(2, 0, 1, 3).reshape(M, 8, M)).astype(bf)

    in_maps = []
    for c in range(NCORES):
        in_maps.append({"etab": etab_np, "idxa": idx_maps[c][0],
                        "idxb": idx_maps[c][1], "wih": wih_sb,
                        "whh": whh_sb})
    return in_maps


def _run_A(inputs):
    from concourse.bass_utils import run_bass_kernel_spmd
    if "nc" not in _cacheA:
        _cacheA["nc"] = _build_A()
    nc = _cacheA["nc"]
    in_maps = _prep_A(inputs)
    res = run_bass_kernel_spmd(nc, in_maps, list(range(NCORES)))
    hid = np.zeros((S + 1, D2), np.float32)
    for c, (g0, nreal) in enumerate(_core_seq_ranges()):
        hT = res.results[c]["hidT"].view(bf).astype(np.float32)  # [200, BP]
        hid[g0:g0 + nreal] = hT[:, :nreal].T
    return hid


# ---------------------------------------------------------------- launch B

def _build_B(key):
    """key = (n_pad, sp, nbr, signs, aligned) — branch-structure parameters."""
    n_pad, sp, nbr, signs, aligned = key
    from contextlib import ExitStack
    import concourse.bacc as bacc
    import concourse.tile as tile
    import concourse.mybir as mybir
    from concourse.masks import make_identity

    dt = mybir.dt
    AF = mybir.ActivationFunctionType
    KCH = sp // 128               # s-tiles per stripe
    TT = n_pad // 128             # key tiles
    assert TT * 128 == n_pad and KCH * 128 == sp and sp <= 512
    assert H * TT <= 128, "Z layout requires H*TT <= 128"
    NROW = sp * D2                # feat rows per stripe
    NSC = (NROW + 2047) // 2048   # feat super-chunks
    scale = 1.0 / float(np.sqrt(np.float32(D2)))

    nc = bacc.Bacc("TRN2", target_bir_lowering=False, debug=False,
                   num_devices=NCORES)
    xf = [nc.dram_tensor(f"xf{b}", [128, 2, n_pad], dt.bfloat16,
                         kind="ExternalInput").ap() for b in range(nbr)]
    xq = [nc.dram_tensor(f"xq{b}", [128, 2, sp], dt.bfloat16,
                         kind="ExternalInput").ap() for b in range(nbr)]
    qkv = nc.dram_tensor("qkv", [128, H, 2, 3, D2], dt.bfloat16,
                         kind="ExternalInput").ap()
    cwb = nc.dram_tensor("cwb", [128, 16, D2], dt.bfloat16,
                         kind="ExternalInput").ap()
    cwf = nc.dram_tensor("cwf", [128, 16, D2], dt.float32,
                         kind="ExternalInput").ap()
    cbias = nc.dram_tensor("cbias", [M, 2], dt.float32,
                           kind="ExternalInput").ap()
    vmask = nc.dram_tensor("vmask", [128, nbr, 128], dt.float32,
                           kind="ExternalInput").ap()
    vbias = nc.dram_tensor("vbias", [128, H, D2], dt.float32,
                           kind="ExternalInput").ap()
    smask = nc.dram_tensor("smask", [1, nbr, sp], dt.float32,
                           kind="ExternalInput").ap()
    ftd = nc.dram_tensor("ftd", [NSC, 128, 16, D2], dt.float32,
                         kind="ExternalInput").ap()
    fpart = nc.dram_tensor("fpart", [nbr, D2], dt.float32,
                           kind="ExternalOutput").ap()

    def head_scores(pP, pT, nc, xf_t, xq_t, k_sb, q_sb, qkv_t, h, b, emit_e):
        """Compute scores^T tiles for head h; yields (tt, psum_tile)."""
        for m in range(2):
            for tchunk in range(n_pad // 512):
                psk = pP.tile([128, 512], dt.float32, tag="pbig", name="psk")
                for kc in range(2):
                    nc.tensor.matmul(
                        psk[0:M, :],
                        lhsT=qkv_t[:, h, kc, 1, M * m:M * (m + 1)],
                        rhs=xf_t[:, kc, 512 * tchunk:512 * (tchunk + 1)],
                        start=(kc == 0), stop=(kc == 1))
                nc.scalar.copy(
                    k_sb[:, m, 512 * tchunk:512 * (tchunk + 1)], psk[0:M, :])
        for m in range(2):
            psq = pP.tile([128, 512], dt.float32, tag="pbig", name="psq")
            for kc in range(2):
                nc.tensor.matmul(
                    psq[0:M, 0:sp], lhsT=qkv_t[:, h, kc, 0, M * m:M * (m + 1)],
                    rhs=xq_t[:, kc, :], start=(kc == 0), stop=(kc == 1))
            nc.scalar.copy(q_sb[:, m, :], psq[0:M, 0:sp])
        for tt in range(TT):
            pss = pP.tile([128, 512], dt.float32, tag="pbig", name="pss")
            for m in range(2):
                nc.tensor.matmul(
                    pss[:, 0:sp], lhsT=k_sb[:, m, 128 * tt:128 * (tt + 1)],
                    rhs=q_sb[:, m, :], start=(m == 0), stop=(m == 1))
            yield tt, pss

    with tile.TileContext(nc) as tc, ExitStack() as ctx:
        pC = ctx.enter_context(tc.tile_pool(name="const", bufs=1))
        pB = ctx.enter_context(tc.tile_pool(name="big", bufs=1))
        pT = ctx.enter_context(tc.tile_pool(name="tmp", bufs=2))
        pF = ctx.enter_context(tc.tile_pool(name="ftst", bufs=4))
        pP = ctx.enter_context(tc.tile_pool(name="ps", bufs=4, space="PSUM"))
        pPacc = ctx.enter_context(tc.tile_pool(name="psa", bufs=1,
                                               space="PSUM"))
        pD = ctx.enter_context(tc.tile_pool(name="dram", bufs=1, space="DRAM"))

        cw_b = pC.tile([128, 16, D2], dt.bfloat16)
        nc.sync.dma_start(cw_b[:], cwb[:])
        cw_f = pC.tile([128, 16, D2], dt.float32)
        nc.sync.dma_start(cw_f[:], cwf[:])
        cb_t = pC.tile([M, 2], dt.float32)
        nc.sync.dma_start(cb_t[:], cbias[:])
        vm_t = pC.tile([128, nbr, 128], dt.float32)
        nc.sync.dma_start(vm_t[:], vmask[:])
        vb_t = pC.tile([128, H, D2], dt.float32)
        nc.sync.dma_start(vb_t[:], vbias[:])
        ones_t = pC.tile([128, 1], dt.bfloat16)
        nc.vector.memset(ones_t[:], 1.0)
        sm_t = None
        if not aligned:
            sm_t = pC.tile([1, nbr, sp], dt.float32)
            nc.sync.dma_start(sm_t[:], smask[:])
        qkv_t = pC.tile([128, H, 2, 3, D2], dt.bfloat16)
        nc.sync.dma_start(qkv_t[:], qkv[:])
        idn = pC.tile([128, 128], dt.float32)
        make_identity(nc, idn[:])

        mht_d = pD.tile([NSC * 2048, nbr], dt.float32)
        zin_d = pD.tile([nbr * 4, 128, 2 * TT], dt.float32)
        zout_d = pD.tile([nbr * 4, 128, 2 * TT], dt.float32)

        fps0 = pPacc.tile([M, nbr], dt.float32)  # feat acc, o in [0,100)
        fps1 = pPacc.tile([M, nbr], dt.float32)  # feat acc, o in [100,200)

        xf_ts, xq_ts = [], []
        for b in range(nbr):
            xf_t = pB.tile([128, 2, n_pad], dt.bfloat16, tag=f"xf{b}")
            nc.sync.dma_start(xf_t[:], xf[b][:])
            xf_ts.append(xf_t)
            xq_t = pB.tile([128, 2, sp], dt.bfloat16, tag=f"xq{b}")
            nc.sync.dma_start(xq_t[:], xq[b][:])
            xq_ts.append(xq_t)

        for b in range(nbr):
            out_sb = pB.tile([128, 16, sp], dt.bfloat16, tag=f"o{b}",
                             name=f"out{b}")
            a_sb = pB.tile([128, 16, 1], dt.float32, tag=f"a{b}",
                           name=f"asb{b}")
            nc.vector.memset(a_sb[:], 0.0)

            def stage1(h, em_all, slot):
                k_sb = pT.tile([M, 2, n_pad], dt.bfloat16, tag="k1",
                               name="ksb", bufs=3)
                q_sb = pT.tile([M, 2, sp], dt.bfloat16, tag="q1", name="qsb")
                zp_sb = pT.tile([128, TT], dt.float32, tag="zp", name="zp")
                for tt, pss in head_scores(pP, pT, nc, xf_ts[b], xq_ts[b],
                                           k_sb, q_sb, qkv_t, h, b, True):
                    e_t = pT.tile([128, sp], dt.float32, tag="e1", name="et")
                    zcol = zp_sb[:, tt:tt + 1]
                    if aligned:
                        nc.scalar.activation(
                            e_t[:], pss[:, 0:sp], AF.Exp,
                            scale=float(signs[b]) * scale, accum_out=zcol)
                    else:
                        nc.scalar.activation(
                            e_t[:], pss[:, 0:sp], AF.Exp,
                            scale=float(signs[b]) * scale)
                        ez = pT.tile([128, sp], dt.float32, tag="ez1",
                                     name="ez")
                        nc.vector.tensor_mul(
                            ez[:], e_t[:],
                            sm_t[0:1, b, :].to_broadcast([128, sp]))
                        nc.vector.reduce_sum(zcol, ez[:],
                                             axis=mybir.AxisListType.X)
                    nc.vector.tensor_scalar_add(
                        em_all[:, slot * TT + tt, :], e_t[:], -1.0)
                nc.gpsimd.dma_start(
                    zin_d[4 * b + h // 2, :, TT * (h % 2):TT * (h % 2 + 1)],
                    zp_sb[:])

            def stage2(h, em_all, slot):
                zall = pT.tile([128, TT], dt.float32, tag="zall", name="zall")
                nc.gpsimd.dma_start(
                    zall[:],
                    zout_d[4 * b + h // 2, :, TT * (h % 2):TT * (h % 2 + 1)])
                zr = pT.tile([128, TT], dt.float32, tag="zr", name="zr")
                nc.vector.reciprocal(zr[:], zall[:])
                nc.vector.tensor_mul(
                    zr[:], zr[:], vm_t[:, b, TT * h:TT * (h + 1)])
                vtmp = pT.tile([128, TT, D2], dt.float32, tag="vtmp",
                               name="vtmp", bufs=1)
                vp_sb = pT.tile([128, TT, 256], dt.bfloat16, tag="vp",
                                name="vpsb")
                nc.vector.memset(vp_sb[:], 0.0)
                psa0 = pP.tile([128, 1], dt.float32, tag="psmall",
                               name="psa0", bufs=2)
                psa1 = pP.tile([128, 1], dt.float32, tag="psmall",
                               name="psa1", bufs=2)
                for tt in range(TT):
                    psv = pP.tile([128, 512], dt.float32, tag="pbig",
                                  name="psv")
                    for kc in range(2):
                        nc.tensor.matmul(
                            psv[:, 0:D2],
                            lhsT=xf_ts[b][:, kc, 128 * tt:128 * (tt + 1)],
                            rhs=qkv_t[:, h, kc, 2, :],
                            start=(kc == 0), stop=(kc == 1))
                    nc.vector.tensor_tensor(
                        out=vtmp[:, tt, :], in0=psv[:, 0:D2],
                        in1=vb_t[:, h, :], op=mybir.AluOpType.add)
                    nc.vector.tensor_scalar_mul(
                        vp_sb[:, tt, 0:D2], vtmp[:, tt, :],
                        zr[:, tt:tt + 1])
                    nc.tensor.matmul(
                        psa0[:, :], lhsT=vtmp[:, tt, 0:128],
                        rhs=zr[:, tt:tt + 1],
                        start=(tt == 0), stop=(tt == TT - 1))
                    nc.tensor.matmul(
                        psa1[0:D2 - 128, :], lhsT=vtmp[:, tt, 128:D2],
                        rhs=zr[:, tt:tt + 1],
                        start=(tt == 0), stop=(tt == TT - 1))
                nc.vector.tensor_copy(a_sb[0:128, 2 * h, :], psa0[:, :])
                nc.vector.tensor_copy(a_sb[0:D2 - 128, 2 * h + 1, :],
                                      psa1[0:D2 - 128, :])
                for j in range(2):
                    pso = pP.tile([128, 512], dt.float32, tag="pbig",
                                  name="pso")
                    for tt in range(TT):
                        nc.tensor.matmul(
                            pso[:, 0:sp],
                            lhsT=vp_sb[:, tt, 128 * j:128 * (j + 1)],
                            rhs=em_all[:, slot * TT + tt, :],
                            start=(tt == 0), stop=(tt == TT - 1))
                    nc.vector.tensor_copy(out_sb[:, 2 * h + j, :],
                                          pso[:, 0:sp])

            pending = None
            for p in range(H // 2):
                em_all = pT.tile([128, 2 * TT, sp], dt.bfloat16, tag="em2",
                                 name="em2")
                stage1(2 * p, em_all, 0)
                stage1(2 * p + 1, em_all, 1)
                nc.gpsimd.collective_compute(
                    "AllReduce", mybir.AluOpType.add,
                    replica_groups=[list(range(NCORES))],
                    ins=[zin_d[4 * b + p].opt()],
                    outs=[zout_d[4 * b + p].opt()])
                if pending is not None:
                    stage2(2 * p - 2, pending, 0)
                    stage2(2 * p - 1, pending, 1)
                pending = em_all
            stage2(H - 2, pending, 0)
            stage2(H - 1, pending, 1)

            # mh^T = cw^T @ out (+ cw^T @ A + concat_b as f32 bias)
            mh_sb = pT.tile([M, 2, sp], dt.float32, tag="mh", bufs=1)
            for bc in range(2):
                psm1 = pP.tile([128, 1], dt.float32, tag="psmall", name="psm1", bufs=2)
                for k in range(16):
                    nc.tensor.matmul(
                        psm1[0:M, :], lhsT=cw_f[:, k, M * bc:M * (bc + 1)],
                        rhs=a_sb[:, k, :], start=(k == 0), stop=(k == 15))
                mha = pT.tile([M, 1], dt.float32, tag="mha")
                nc.vector.tensor_tensor(
                    out=mha[:], in0=psm1[0:M, :], in1=cb_t[:, bc:bc + 1],
                    op=mybir.AluOpType.add)
                psm = pP.tile([128, 512], dt.float32, tag="pbig", name="psm")
                for k in range(16):
                    nc.tensor.matmul(
                        psm[0:M, 0:sp], lhsT=cw_b[:, k, M * bc:M * (bc + 1)],
                        rhs=out_sb[:, k, :], start=(k == 0), stop=(k == 15))
                nc.scalar.activation(mh_sb[:, bc, :], psm[0:M, 0:sp],
                                     AF.Identity, bias=mha[:])

            # transpose mh^T -> token-major, store to MHT dram column b
            mtk = pT.tile([128, KCH, D2], dt.float32, tag="mtk", bufs=1)
            for bc in range(2):
                for k in range(KCH):
                    pst = pP.tile([128, 512], dt.float32, tag="pbig",
                                  name="pst")
                    nc.tensor.transpose(
                        pst[:, 0:M], mh_sb[:, bc, 128 * k:128 * (k + 1)],
                        idn[0:M, 0:M])
                    nc.vector.tensor_copy(mtk[:, k, M * bc:M * (bc + 1)],
                                          pst[:, 0:M])
            mview = mht_d[0:KCH * 128 * D2, :].rearrange(
                "(k p c) r -> p k c r", p=128, c=D2)
            nc.sync.dma_start(mview[:, :, :, b], mtk[:])

        if NSC * 2048 > NROW:
            ztail = pT.tile([128, nbr], dt.float32, tag="ztail")
            nc.vector.memset(ztail[:], 0.0)
            tail = mht_d[:].rearrange("(c u p) r -> p c u r", c=NSC, u=16)
            for u in range(16):
                row0 = (NSC - 1) * 2048 + 128 * u
                if row0 >= NROW:
                    nc.sync.dma_start(tail[:, NSC - 1, u, :], ztail[:])

        # feat contraction: fpart^T += featT_chunk.T @ MHT_chunk
        for c in range(NSC):
            ft_t = pF.tile([128, 16, D2], dt.float32, tag="ft")
            nc.sync.dma_start(ft_t[:], ftd[c])
            mh_t = pF.tile([128, 16, nbr], dt.float32, tag="mht")
            nc.gpsimd.dma_start(
                mh_t[:],
                mht_d[:].rearrange("(c u p) r -> p c u r",
                                   c=NSC, u=16)[:, c, :, :])
            for u in range(16):
                nc.tensor.matmul(
                    fps0[:, :], lhsT=ft_t[:, u, 0:M], rhs=mh_t[:, u, :],
                    start=(c == 0 and u == 0),
                    stop=(c == NSC - 1 and u == 15))
                nc.tensor.matmul(
                    fps1[:, :], lhsT=ft_t[:, u, M:D2], rhs=mh_t[:, u, :],
                    start=(c == 0 and u == 0),
                    stop=(c == NSC - 1 and u == 15))
        ot = pT.tile([M, 2, nbr], dt.float32, tag="ot")
        nc.vector.tensor_copy(ot[:, 0, :], fps0[:, :])
        nc.vector.tensor_copy(ot[:, 1, :], fps1[:, :])
        nc.sync.dma_start(
            fpart[:].rearrange("r (k o) -> o k r", k=2), ot[:])

    nc.compile()
    return nc


def _prep_B(inputs, branches, n_pad, sp, aligned):
    """branches: list of (X_sorted [S, D2] f32, n_valid, sign)."""
    nbr = len(branches)
    TT = n_pad // 128
    qkv_np = np.zeros((2, 3, H, 128, D2), np.float32)
    for w_i, wkey, bkey in ((0, "qw", "qb"), (1, "kw", "kb"), (2, "vw", "vb")):
        wa = np.asarray(inputs[wkey], np.float32)   # [H, 200, 200] (out, in)
        ba = np.asarray(inputs[bkey], np.float32)   # [H, 200]
        for h in range(H):
            wT = np.zeros((256, D2), np.float32)
            wT[:D2] = wa[h].T
            if w_i != 2:
                wT[D2] = ba[h]   # v bias added in f32 on device instead
            qkv_np[0, w_i, h] = wT[:128]
            qkv_np[1, w_i, h] = wT[128:]
    vbias_np = np.ascontiguousarray(np.broadcast_to(
        np.asarray(inputs["vb"], np.float32)[None, :, :], (128, H, D2)))
    qkv_sb = np.ascontiguousarray(
        qkv_np.transpose(3, 2, 0, 1, 4)).astype(bf)  # [128, H, 2, 3, 200]

    cw = np.asarray(inputs["concat_w"], np.float32)  # [200, 1600]
    cwp = np.zeros((2048, D2), np.float32)
    for h in range(H):
        cwp[256 * h:256 * h + D2] = cw[:, D2 * h:D2 * (h + 1)].T
    cw_sb = np.ascontiguousarray(cwp.reshape(16, 128, D2).transpose(1, 0, 2))
    cbias_np = np.ascontiguousarray(
        np.asarray(inputs["concat_b"], np.float32).reshape(2, M).T)

    xf_list = []
    vm = np.zeros((128, nbr, 128), np.float32)
    for b, (Xs, n, sign) in enumerate(branches):
        xfp = np.zeros((256, n_pad), np.float32)
        xfp[:D2, :n] = Xs[:n].T
        xfp[D2, :n] = 1.0
        xf_list.append(np.ascontiguousarray(
            xfp.reshape(2, 128, n_pad).transpose(1, 0, 2)).astype(bf))
        for tt in range(TT):
            valid = (np.arange(128) + 128 * tt) < n
            for h in range(H):
                vm[:, b, h * TT + tt] = valid

    featT = np.ascontiguousarray(np.asarray(inputs["feat_w"], np.float32).T)
    NROW = sp * D2
    NSC = (NROW + 2047) // 2048

    in_maps = []
    for c in range(NCORES):
        s0 = c * sp
        m = {"qkv": qkv_sb, "cwb": cw_sb.astype(bf), "cwf": cw_sb,
             "cbias": cbias_np, "vmask": vm, "vbias": vbias_np}
        sm = np.zeros((1, nbr, sp), np.float32)
        for b, (Xs, n, sign) in enumerate(branches):
            xq = np.zeros((256, sp), np.float32)
            valid_cols = max(0, min(sp, n - s0))
            if valid_cols > 0:
                xq[:D2, :valid_cols] = Xs[s0:s0 + valid_cols].T
                xq[D2, :valid_cols] = 1.0
                sm[0, b, :valid_cols] = 1.0
            m[f"xf{b}"] = xf_list[b]
            m[f"xq{b}"] = np.ascontiguousarray(
                xq.reshape(2, 128, sp).transpose(1, 0, 2)).astype(bf)
        m["smask"] = sm
        stripe = np.zeros((NSC * 2048, D2), np.float32)
        r0 = s0 * D2
        rows = max(0, min(NROW, featT.shape[0] - r0))
        if rows > 0:
            stripe[:rows] = featT[r0:r0 + rows]
        m["ftd"] = np.ascontiguousarray(
            stripe.reshape(NSC, 16, 128, D2).transpose(0, 2, 1, 3))
        in_maps.append(m)
    return in_maps


def _run_B(inputs, branches):
    from concourse.bass_utils import run_bass_kernel_spmd
    nmax = max(n for _, n, _ in branches)
    sp = -(-nmax // (NCORES * 128)) * 128
    n_pad = sp * NCORES
    aligned = all(n == n_pad for _, n, _ in branches)
    signs = tuple(sign for _, _, sign in branches)
    key = (n_pad, sp, len(branches), signs, aligned)
    if key not in _cacheB:
        _cacheB[key] = _build_B(key)
    nc = _cacheB[key]
    in_maps = _prep_B(inputs, branches, n_pad, sp, aligned)
    res = run_bass_kernel_spmd(nc, in_maps, list(range(NCORES)))
    parts = np.stack([res.results[c]["fpart"] for c in range(NCORES)])
    return parts.sum(axis=0)  # [nbr, 200]


# ------------------------------------------------------------------ driver

def kernel(**inputs):
    hid = _run_A(inputs)

    head, sents = hid[0], hid[1:]
    u = head.astype(np.float32) @ np.asarray(inputs["sim_w"], np.float32)
    logits = sents @ u + np.asarray(inputs["sim_b"], np.float32)[0]
    sig = (1.0 / (1.0 + np.exp(-logits))).astype(np.float32)
    e = np.exp(sig - sig.max())
    prob = (e / e.sum()).astype(np.float32)
    attend = (prob[:, None] * sents).astype(np.float32)

    mask = sig >= 0.5
    n_high = int(mask.sum())
    n_low = S - n_high
    feat_b = np.asarray(inputs["feat_b"], np.float32)

    branches = []
    slot = {}
    if n_high > 0:
        order = np.argsort(np.where(mask, 0, 1), kind="stable")
        slot[0] = len(branches)
        branches.append((attend[order], n_high, 1.0))
    if n_low > 0:
        order = np.argsort(np.where(mask, 1, 0), kind="stable")
        slot[1] = len(branches)
        branches.append((attend[order], n_low, -1.0))

    out = np.zeros((2, D2), np.float32)
    if branches:
        parts = _run_B(inputs, branches)
        for r in range(2):
            out[r] = (parts[slot[r]] + feat_b) if r in slot else feat_b
    else:
        out[0] = feat_b
        out[1] = feat_b
    return out.astype(np.float32)


# revision 41
# speedup vs baseline: 1.5535x; 1.0003x over previous
"""Bass/Tile TRN2 kernel for nn_DocLSTM (BiLSTM doc encoder + two-branch
block-sparse attention + feature projection), SPMD over 8 NeuronCores.

Structure:
  Launch A: embedding gather (dma_gather transpose mode, int16 two-pass
            split table) + batched BiLSTM over 2049 sequences sharded by
            sequence across the 8 cores -> hid [2049, 200].
  Host glue: similarity logits, sigmoid, softmax, attend matrix, mask,
            stable-partition orders (tiny O(S*D2) work).
  Launch B: per-branch multihead attention (softmax over the *query* axis,
            computed exactly via per-key normalizers Z_t) sharded by
            sentence stripe, plus the 327MB feat_w contraction sharded by
            the same stripes.  Two phases: phase 1 computes the Z_t
            partials (scores+exp only), an AllReduce combines them, and
            phase 2 recomputes scores and applies  out = A + (E-1) @ V/Z
            so the bf16 matmul never sees the near-constant part of E.
"""

import numpy as np
import ml_dtypes

bf = ml_dtypes.bfloat16

V, D, M, H, S, W, D2 = 50000, 300, 100, 8, 2048, 64, 200
NCORES = 8
BP = 264                  # padded sequences per core (max real = 257)
TOK = W * BP              # gather slots per core (16896, %128==0)
NCH = 16                  # X chunk tiles (4 steps each)
CHTOK = TOK // NCH        # 2112 tokens per X chunk
GSUB = 768                # idxs per dma_gather call (HW ring limit < 1024)
NGS = TOK // GSUB         # 22 gather calls per pass
EW = 384                  # padded embedding row (bf16 -> 768B, %256==0)
SPLIT = 32768             # int16 index limit for dma_gather
VROWS = V + 1             # + zero row for the pass-B redirect
GPERM = [0, 1, 3, 2]      # gate slot -> pytorch row block (i, f, o, g)

_cacheA = {}
_cacheB = {}


def _wrap_idx(ids):
    """Token list -> dma_gather index tile [128, n/16] int16.

    Position i lives at partition i%16, column i//16; the 16-partition block
    is replicated to all 128 partitions (one copy per GPSIMD core).
    """
    n = len(ids)
    out = np.zeros((16, n // 16), np.int16)
    out[np.arange(n) % 16, np.arange(n) // 16] = ids
    return np.tile(out, (8, 1))


def _core_seq_ranges():
    """Global sequence index ranges per core. Sequence 0 is the head (rsent)."""
    ranges = []
    start = 0
    for c in range(NCORES):
        nreal = 257 if c == 0 else 256
        ranges.append((start, nreal))
        start += nreal
    return ranges


# ---------------------------------------------------------------- launch A

def _build_A():
    from contextlib import ExitStack
    import concourse.bacc as bacc
    import concourse.tile as tile
    import concourse.mybir as mybir

    dt = mybir.dt
    AF = mybir.ActivationFunctionType
    nc = bacc.Bacc("TRN2", target_bir_lowering=False, debug=False,
                   num_devices=NCORES)
    etab = nc.dram_tensor("etab", [VROWS, EW], dt.bfloat16,
                          kind="ExternalInput").ap()
    idxa = nc.dram_tensor("idxa", [128, TOK // 16], dt.int16,
                          kind="ExternalInput").ap()
    idxb = nc.dram_tensor("idxb", [128, TOK // 16], dt.int16,
                          kind="ExternalInput").ap()
    wih = nc.dram_tensor("wih", [128, 24, M], dt.bfloat16,
                         kind="ExternalInput").ap()
    whh = nc.dram_tensor("whh", [M, 8, M], dt.bfloat16,
                         kind="ExternalInput").ap()
    hidT = nc.dram_tensor("hidT", [D2, BP], dt.bfloat16,
                          kind="ExternalOutput").ap()

    with tile.TileContext(nc) as tc, ExitStack() as ctx:
        pC = ctx.enter_context(tc.tile_pool(name="const", bufs=1))
        pX = ctx.enter_context(tc.tile_pool(name="xt", bufs=1))
        pG = ctx.enter_context(tc.tile_pool(name="gst", bufs=2))
        pSt = ctx.enter_context(tc.tile_pool(name="state", bufs=1))
        pA = ctx.enter_context(tc.tile_pool(name="act", bufs=2))
        pP = ctx.enter_context(tc.tile_pool(name="ps", bufs=1, space="PSUM"))

        idxa_t = pC.tile([128, TOK // 16], dt.int16)
        nc.sync.dma_start(idxa_t[:], idxa[:])
        idxb_t = pC.tile([128, TOK // 16], dt.int16)
        nc.sync.dma_start(idxb_t[:], idxb[:])
        wih_t = pC.tile([128, 24, M], dt.bfloat16)
        nc.sync.dma_start(wih_t[:], wih[:])
        whh_t = pC.tile([M, 8, M], dt.bfloat16)
        nc.sync.dma_start(whh_t[:], whh[:])

        X = [pX.tile([128, 3, CHTOK], dt.bfloat16, tag=f"x{c}",
                     name=f"xchunk{c}") for c in range(NCH)]
        jorder = []
        for i in range((NGS + 1) // 2):
            jorder.append(i)
            if NGS - 1 - i != i:
                jorder.append(NGS - 1 - i)
        for j in jorder:
            ga = pG.tile([128, 3, GSUB], dt.bfloat16, tag="ga")
            gb = pG.tile([128, 3, GSUB], dt.bfloat16, tag="gb")
            isl = slice(j * (GSUB // 16), (j + 1) * (GSUB // 16))
            nc.gpsimd.dma_gather(
                out_ap=ga[:], in_ap=etab[0:SPLIT, :], idxs_ap=idxa_t[:, isl],
                num_idxs=GSUB, num_idxs_reg=GSUB, elem_size=EW,
                transpose=True)
            nc.gpsimd.dma_gather(
                out_ap=gb[:], in_ap=etab[SPLIT:VROWS, :],
                idxs_ap=idxb_t[:, isl],
                num_idxs=GSUB, num_idxs_reg=GSUB, elem_size=EW,
                transpose=True)
            # add into the X chunk tiles this sub-range spans
            lo = j * GSUB
            while lo < (j + 1) * GSUB:
                c = lo // CHTOK
                hi = min((j + 1) * GSUB, (c + 1) * CHTOK)
                s0 = lo - j * GSUB
                nc.vector.tensor_add(
                    X[c][:, :, lo - c * CHTOK:hi - c * CHTOK],
                    ga[:, :, s0:s0 + hi - lo], gb[:, :, s0:s0 + hi - lo])
                lo = hi

        h_t, c_t = [], []
        for d in range(2):
            ht = pSt.tile([M, BP], dt.bfloat16, tag=f"h{d}", name=f"h{d}")
            nc.vector.memset(ht[:], 0.0)
            h_t.append(ht)
            ct = pSt.tile([M, BP], dt.float32, tag=f"c{d}", name=f"c{d}")
            nc.vector.memset(ct[:], 0.0)
            c_t.append(ct)

        for t in range(W):
            for d in range(2):
                tok = t if d == 0 else W - 1 - t
                ch, off = divmod(tok, W // NCH)
                off *= BP
                # per-gate psum tiles; order g,i,f,o so the c-chain
                # (t1 = sig_i*tanh_g) can start as early as possible
                gact = {}
                for s in (2, 0, 1, 3):
                    gp = pP.tile([M, BP], dt.float32, tag=f"ps{d}g{s}",
                                 name=f"gp{d}{s}")
                    for kc in range(3):
                        nc.tensor.matmul(
                            gp[:, :],
                            lhsT=wih_t[:, d * 12 + s * 3 + kc, :],
                            rhs=X[ch][:, kc, off:off + BP],
                            start=(kc == 0), stop=False)
                    nc.tensor.matmul(
                        gp[:, :],
                        lhsT=whh_t[:, d * 4 + s, :], rhs=h_t[d][:],
                        start=False, stop=True)
                    av = pA.tile([M, BP], dt.float32, tag=f"ac{d}{s}",
                                 name=f"av{d}{s}")
                    nc.scalar.activation(
                        av[:], gp[:, :],
                        AF.Tanh if s == 2 else AF.Sigmoid)
                    gact[s] = av
                t1 = pA.tile([M, BP], dt.float32, tag=f"t1{d}")
                nc.vector.tensor_mul(t1[:], gact[0][:], gact[2][:])
                nc.vector.tensor_mul(c_t[d][:], c_t[d][:], gact[1][:])
                nc.vector.tensor_add(c_t[d][:], c_t[d][:], t1[:])
                tanhc = pA.tile([M, BP], dt.float32, tag=f"tc{d}")
                nc.scalar.activation(tanhc[:], c_t[d][:], AF.Tanh)
                nc.vector.tensor_mul(h_t[d][:], gact[3][:], tanhc[:])

        nc.sync.dma_start(hidT[0:M, :], h_t[0][:])
        nc.sync.dma_start(hidT[M:D2, :], h_t[1][:])

    nc.compile()
    return nc


def _prep_A(inputs):
    emb = np.ascontiguousarray(inputs["emb"], dtype=np.float32)
    emb_pad = np.zeros((VROWS, EW), np.float32)
    emb_pad[:V, :D] = emb
    emb_pad[:, D] = 0.5       # bias lane: two gather passes sum to 1.0
    emb_pad[V, :D] = 0.0      # pass-B redirect row
    etab_np = emb_pad.astype(bf)

    tok_all = np.concatenate(
        [np.asarray(inputs["rsent"], np.int64)[None, :],
         np.asarray(inputs["body_sents"], np.int64)], axis=0)  # [2049, 64]

    idx_maps = []
    for c, (g0, nreal) in enumerate(_core_seq_ranges()):
        grid = np.zeros((W, BP), np.int64)
        grid[:, :nreal] = tok_all[g0:g0 + nreal].T  # [W, nreal]
        ids = grid.reshape(-1)
        ida = np.where(ids < SPLIT, ids, 0).astype(np.int16)
        idb = np.where(ids >= SPLIT, ids - SPLIT,
                       VROWS - 1 - SPLIT).astype(np.int16)
        idx_maps.append((_wrap_idx(ida), _wrap_idx(idb)))

    wih_np = np.zeros((2, 4, 3, 128, M), np.float32)
    whh_np = np.zeros((2, 4, M, M), np.float32)
    for d, (w_ih, w_hh, b_ih, b_hh) in enumerate([
            (inputs["w_ih_f"], inputs["w_hh_f"], inputs["b_ih_f"], inputs["b_hh_f"]),
            (inputs["w_ih_b"], inputs["w_hh_b"], inputs["b_ih_b"], inputs["b_hh_b"])]):
        btot = (np.asarray(b_ih, np.float32) + np.asarray(b_hh, np.float32))
        wT = np.zeros((EW, 4 * M), np.float32)
        wT[:D, :] = np.asarray(w_ih, np.float32).T
        wT[D, :] = btot
        for s, blk in enumerate(GPERM):
            for kc in range(3):
                wih_np[d, s, kc] = wT[128 * kc:128 * (kc + 1),
                                      M * blk:M * (blk + 1)]
            whh_np[d, s] = np.asarray(w_hh, np.float32).T[:, M * blk:M * (blk + 1)]
    wih_sb = np.ascontiguousarray(
        wih_np.transpose(3, 0, 1, 2, 4).reshape(128, 24, M)).astype(bf)
    whh_sb = np.ascontiguousarray(
        whh_np.transpose(2, 0, 1, 3).reshape(M, 8, M)).astype(bf)

    in_maps = []
    for c in range(NCORES):
        in_maps.append({"etab": etab_np, "idxa": idx_maps[c][0],
                        "idxb": idx_maps[c][1], "wih": wih_sb,
                        "whh": whh_sb})
    return in_maps


def _run_A(inputs):
    from concourse.bass_utils import run_bass_kernel_spmd
    if "nc" not in _cacheA:
        _cacheA["nc"] = _build_A()
    nc = _cacheA["nc"]
    in_maps = _prep_A(inputs)
    res = run_bass_kernel_spmd(nc, in_maps, list(range(NCORES)))
    hid = np.zeros((S + 1, D2), np.float32)
    for c, (g0, nreal) in enumerate(_core_seq_ranges()):
        hT = res.results[c]["hidT"].view(bf).astype(np.float32)  # [200, BP]
        hid[g0:g0 + nreal] = hT[:, :nreal].T
    return hid


# ---------------------------------------------------------------- launch B

def _build_B(key):
    """key = (n_pad, sp, nbr, signs, aligned) — branch-structure parameters."""
    n_pad, sp, nbr, signs, aligned = key
    from contextlib import ExitStack
    import concourse.bacc as bacc
    import concourse.tile as tile
    import concourse.mybir as mybir
    from concourse.masks import make_identity

    dt = mybir.dt
    AF = mybir.ActivationFunctionType
    KCH = sp // 128               # s-tiles per stripe
    TT = n_pad // 128             # key tiles
    assert TT * 128 == n_pad and KCH * 128 == sp and sp <= 512
    assert H * TT <= 128, "Z layout requires H*TT <= 128"
    NROW = sp * D2                # feat rows per stripe
    NSC = (NROW + 2047) // 2048   # feat super-chunks
    scale = 1.0 / float(np.sqrt(np.float32(D2)))

    nc = bacc.Bacc("TRN2", target_bir_lowering=False, debug=False,
                   num_devices=NCORES)
    xf = [nc.dram_tensor(f"xf{b}", [128, 2, n_pad], dt.bfloat16,
                         kind="ExternalInput").ap() for b in range(nbr)]
    xq = [nc.dram_tensor(f"xq{b}", [128, 2, sp], dt.bfloat16,
                         kind="ExternalInput").ap() for b in range(nbr)]
    qkv = nc.dram_tensor("qkv", [128, H, 2, 3, D2], dt.bfloat16,
                         kind="ExternalInput").ap()
    cwb = nc.dram_tensor("cwb", [128, 16, D2], dt.bfloat16,
                         kind="ExternalInput").ap()
    cwf = nc.dram_tensor("cwf", [128, 16, D2], dt.float32,
                         kind="ExternalInput").ap()
    cbias = nc.dram_tensor("cbias", [M, 2], dt.float32,
                           kind="ExternalInput").ap()
    vmask = nc.dram_tensor("vmask", [128, nbr, 128], dt.float32,
                           kind="ExternalInput").ap()
    vbias = nc.dram_tensor("vbias", [128, H, D2], dt.float32,
                           kind="ExternalInput").ap()
    smask = nc.dram_tensor("smask", [1, nbr, sp], dt.float32,
                           kind="ExternalInput").ap()
    ftd = nc.dram_tensor("ftd", [NSC, 128, 16, D2], dt.float32,
                         kind="ExternalInput").ap()
    fpart = nc.dram_tensor("fpart", [nbr, D2], dt.float32,
                           kind="ExternalOutput").ap()

    def head_scores(pP, pT, nc, xf_t, xq_t, k_sb, q_sb, qkv_t, h, b, emit_e):
        """Compute scores^T tiles for head h; yields (tt, psum_tile)."""
        for m in range(2):
            for tchunk in range(n_pad // 512):
                psk = pP.tile([128, 512], dt.float32, tag="pbig", name="psk")
                for kc in range(2):
                    nc.tensor.matmul(
                        psk[0:M, :],
                        lhsT=qkv_t[:, h, kc, 1, M * m:M * (m + 1)],
                        rhs=xf_t[:, kc, 512 * tchunk:512 * (tchunk + 1)],
                        start=(kc == 0), stop=(kc == 1))
                nc.scalar.copy(
                    k_sb[:, m, 512 * tchunk:512 * (tchunk + 1)], psk[0:M, :])
        for m in range(2):
            psq = pP.tile([128, 512], dt.float32, tag="pbig", name="psq")
            for kc in range(2):
                nc.tensor.matmul(
                    psq[0:M, 0:sp], lhsT=qkv_t[:, h, kc, 0, M * m:M * (m + 1)],
                    rhs=xq_t[:, kc, :], start=(kc == 0), stop=(kc == 1))
            nc.scalar.copy(q_sb[:, m, :], psq[0:M, 0:sp])
        for tt in range(TT):
            pss = pP.tile([128, 512], dt.float32, tag="pbig", name="pss")
            for m in range(2):
                nc.tensor.matmul(
                    pss[:, 0:sp], lhsT=k_sb[:, m, 128 * tt:128 * (tt + 1)],
                    rhs=q_sb[:, m, :], start=(m == 0), stop=(m == 1))
            yield tt, pss

    with tile.TileContext(nc) as tc, ExitStack() as ctx:
        pC = ctx.enter_context(tc.tile_pool(name="const", bufs=1))
        pB = ctx.enter_context(tc.tile_pool(name="big", bufs=1))
        pT = ctx.enter_context(tc.tile_pool(name="tmp", bufs=2))
        pF = ctx.enter_context(tc.tile_pool(name="ftst", bufs=4))
        pP = ctx.enter_context(tc.tile_pool(name="ps", bufs=4, space="PSUM"))
        pPacc = ctx.enter_context(tc.tile_pool(name="psa", bufs=1,
                                               space="PSUM"))
        pD = ctx.enter_context(tc.tile_pool(name="dram", bufs=1, space="DRAM"))

        cw_b = pC.tile([128, 16, D2], dt.bfloat16)
        nc.sync.dma_start(cw_b[:], cwb[:])
        cw_f = pC.tile([128, 16, D2], dt.float32)
        nc.sync.dma_start(cw_f[:], cwf[:])
        cb_t = pC.tile([M, 2], dt.float32)
        nc.sync.dma_start(cb_t[:], cbias[:])
        vm_t = pC.tile([128, nbr, 128], dt.float32)
        nc.sync.dma_start(vm_t[:], vmask[:])
        vb_t = pC.tile([128, H, D2], dt.float32)
        nc.sync.dma_start(vb_t[:], vbias[:])
        ones_t = pC.tile([128, 1], dt.bfloat16)
        nc.vector.memset(ones_t[:], 1.0)
        sm_t = None
        if not aligned:
            sm_t = pC.tile([1, nbr, sp], dt.float32)
            nc.sync.dma_start(sm_t[:], smask[:])
        qkv_t = pC.tile([128, H, 2, 3, D2], dt.bfloat16)
        nc.sync.dma_start(qkv_t[:], qkv[:])
        idn = pC.tile([128, 128], dt.float32)
        make_identity(nc, idn[:])

        mht_d = pD.tile([NSC * 2048, nbr], dt.float32)
        zin_d = pD.tile([nbr * 4, 128, 2 * TT], dt.float32)
        zout_d = pD.tile([nbr * 4, 128, 2 * TT], dt.float32)

        fps0 = pPacc.tile([M, nbr], dt.float32)  # feat acc, o in [0,100)
        fps1 = pPacc.tile([M, nbr], dt.float32)  # feat acc, o in [100,200)

        xf_ts, xq_ts = [], []
        for b in range(nbr):
            xf_t = pB.tile([128, 2, n_pad], dt.bfloat16, tag=f"xf{b}")
            nc.sync.dma_start(xf_t[:], xf[b][:])
            xf_ts.append(xf_t)
            xq_t = pB.tile([128, 2, sp], dt.bfloat16, tag=f"xq{b}")
            nc.sync.dma_start(xq_t[:], xq[b][:])
            xq_ts.append(xq_t)

        for b in range(nbr):
            out_sb = pB.tile([128, 16, sp], dt.bfloat16, tag=f"o{b}",
                             name=f"out{b}")
            a_sb = pB.tile([128, 16, 1], dt.float32, tag=f"a{b}",
                           name=f"asb{b}")
            nc.vector.memset(a_sb[:], 0.0)

            def stage1(h, em_all, slot):
                k_sb = pT.tile([M, 2, n_pad], dt.bfloat16, tag="k1",
                               name="ksb", bufs=3)
                q_sb = pT.tile([M, 2, sp], dt.bfloat16, tag="q1", name="qsb")
                zp_sb = pT.tile([128, TT], dt.float32, tag="zp", name="zp")
                for tt, pss in head_scores(pP, pT, nc, xf_ts[b], xq_ts[b],
                                           k_sb, q_sb, qkv_t, h, b, True):
                    e_t = pT.tile([128, sp], dt.float32, tag="e1", name="et")
                    zcol = zp_sb[:, tt:tt + 1]
                    if aligned:
                        nc.scalar.activation(
                            e_t[:], pss[:, 0:sp], AF.Exp,
                            scale=float(signs[b]) * scale, accum_out=zcol)
                    else:
                        nc.scalar.activation(
                            e_t[:], pss[:, 0:sp], AF.Exp,
                            scale=float(signs[b]) * scale)
                        ez = pT.tile([128, sp], dt.float32, tag="ez1",
                                     name="ez")
                        nc.vector.tensor_mul(
                            ez[:], e_t[:],
                            sm_t[0:1, b, :].to_broadcast([128, sp]))
                        nc.vector.reduce_sum(zcol, ez[:],
                                             axis=mybir.AxisListType.X)
                    nc.vector.tensor_scalar_add(
                        em_all[:, slot * TT + tt, :], e_t[:], -1.0)
                nc.gpsimd.dma_start(
                    zin_d[4 * b + h // 2, :, TT * (h % 2):TT * (h % 2 + 1)],
                    zp_sb[:])

            def stage2(h, em_all, slot):
                zall = pT.tile([128, TT], dt.float32, tag="zall", name="zall")
                nc.gpsimd.dma_start(
                    zall[:],
                    zout_d[4 * b + h // 2, :, TT * (h % 2):TT * (h % 2 + 1)])
                zr = pT.tile([128, TT], dt.float32, tag="zr", name="zr")
                nc.vector.reciprocal(zr[:], zall[:])
                nc.vector.tensor_mul(
                    zr[:], zr[:], vm_t[:, b, TT * h:TT * (h + 1)])
                vtmp = pT.tile([128, TT, D2], dt.float32, tag="vtmp",
                               name="vtmp", bufs=1)
                vp_sb = pT.tile([128, TT, 256], dt.bfloat16, tag="vp",
                                name="vpsb")
                nc.vector.memset(vp_sb[:], 0.0)
                psa0 = pP.tile([128, 1], dt.float32, tag="psmall",
                               name="psa0", bufs=2)
                psa1 = pP.tile([128, 1], dt.float32, tag="psmall",
                               name="psa1", bufs=2)
                for tt in range(TT):
                    psv = pP.tile([128, 512], dt.float32, tag="pbig",
                                  name="psv")
                    for kc in range(2):
                        nc.tensor.matmul(
                            psv[:, 0:D2],
                            lhsT=xf_ts[b][:, kc, 128 * tt:128 * (tt + 1)],
                            rhs=qkv_t[:, h, kc, 2, :],
                            start=(kc == 0), stop=(kc == 1))
                    nc.vector.tensor_tensor(
                        out=vtmp[:, tt, :], in0=psv[:, 0:D2],
                        in1=vb_t[:, h, :], op=mybir.AluOpType.add)
                    nc.vector.tensor_scalar_mul(
                        vp_sb[:, tt, 0:D2], vtmp[:, tt, :],
                        zr[:, tt:tt + 1])
                    nc.tensor.matmul(
                        psa0[:, :], lhsT=vtmp[:, tt, 0:128],
                        rhs=zr[:, tt:tt + 1],
                        start=(tt == 0), stop=(tt == TT - 1))
                    nc.tensor.matmul(
                        psa1[0:D2 - 128, :], lhsT=vtmp[:, tt, 128:D2],
                        rhs=zr[:, tt:tt + 1],
                        start=(tt == 0), stop=(tt == TT - 1))
                nc.vector.tensor_copy(a_sb[0:128, 2 * h, :], psa0[:, :])
                nc.vector.tensor_copy(a_sb[0:D2 - 128, 2 * h + 1, :],
                                      psa1[0:D2 - 128, :])
                for j in range(2):
                    pso = pP.tile([128, 512], dt.float32, tag="pbig",
                                  name="pso")
                    for tt in range(TT):
                        nc.tensor.matmul(
                            pso[:, 0:sp],
                            lhsT=vp_sb[:, tt, 128 * j:128 * (j + 1)],
                            rhs=em_all[:, slot * TT + tt, :],
                            start=(tt == 0), stop=(tt == TT - 1))
                    nc.vector.tensor_copy(out_sb[:, 2 * h + j, :],
                                          pso[:, 0:sp])

            pending = None
            for p in range(H // 2):
                em_all = pT.tile([128, 2 * TT, sp], dt.bfloat16, tag="em2",
                                 name="em2")
                stage1(2 * p, em_all, 0)
                stage1(2 * p + 1, em_all, 1)
                nc.gpsimd.collective_compute(
                    "AllReduce", mybir.AluOpType.add,
                    replica_groups=[list(range(NCORES))],
                    ins=[zin_d[4 * b + p].opt()],
                    outs=[zout_d[4 * b + p].opt()])
                if pending is not None:
                    stage2(2 * p - 2, pending, 0)
                    stage2(2 * p - 1, pending, 1)
                pending = em_all
            stage2(H - 2, pending, 0)
            stage2(H - 1, pending, 1)

            # mh^T = cw^T @ out (+ cw^T @ A + concat_b as f32 bias)
            mh_sb = pT.tile([M, 2, sp], dt.float32, tag="mh", bufs=1)
            for bc in range(2):
                psm1 = pP.tile([128, 1], dt.float32, tag="psmall", name="psm1", bufs=2)
                for k in range(16):
                    nc.tensor.matmul(
                        psm1[0:M, :], lhsT=cw_f[:, k, M * bc:M * (bc + 1)],
                        rhs=a_sb[:, k, :], start=(k == 0), stop=(k == 15))
                mha = pT.tile([M, 1], dt.float32, tag="mha")
                nc.vector.tensor_tensor(
                    out=mha[:], in0=psm1[0:M, :], in1=cb_t[:, bc:bc + 1],
                    op=mybir.AluOpType.add)
                psm = pP.tile([128, 512], dt.float32, tag="pbig", name="psm")
                for k in range(16):
                    nc.tensor.matmul(
                        psm[0:M, 0:sp], lhsT=cw_b[:, k, M * bc:M * (bc + 1)],
                        rhs=out_sb[:, k, :], start=(k == 0), stop=(k == 15))
                nc.scalar.activation(mh_sb[:, bc, :], psm[0:M, 0:sp],
                                     AF.Identity, bias=mha[:])

            # transpose mh^T -> token-major, store to MHT dram column b
            mtk = pT.tile([128, KCH, D2], dt.float32, tag="mtk", bufs=1)
            for bc in range(2):
                for k in range(KCH):
                    pst = pP.tile([128, 512], dt.float32, tag="pbig",
                                  name="pst")
                    nc.tensor.transpose(
                        pst[:, 0:M], mh_sb[:, bc, 128 * k:128 * (k + 1)],
                        idn[0:M, 0:M])
                    nc.vector.tensor_copy(mtk[:, k, M * bc:M * (bc + 1)],
                                          pst[:, 0:M])
            mview = mht_d[0:KCH * 128 * D2, :].rearrange(
                "(k p c) r -> p k c r", p=128, c=D2)
            nc.sync.dma_start(mview[:, :, :, b], mtk[:])

        if NSC * 2048 > NROW:
            ztail = pT.tile([128, nbr], dt.float32, tag="ztail")
            nc.vector.memset(ztail[:], 0.0)
            tail = mht_d[:].rearrange("(c u p) r -> p c u r", c=NSC, u=16)
            for u in range(16):
                row0 = (NSC - 1) * 2048 + 128 * u
                if row0 >= NROW:
                    nc.sync.dma_start(tail[:, NSC - 1, u, :], ztail[:])

        # feat contraction: fpart^T += featT_chunk.T @ MHT_chunk
        for c in range(NSC):
            ft_t = pF.tile([128, 16, D2], dt.float32, tag="ft")
            nc.sync.dma_start(ft_t[:], ftd[c])
            mh_t = pF.tile([128, 16, nbr], dt.float32, tag="mht")
            nc.gpsimd.dma_start(
                mh_t[:],
                mht_d[:].rearrange("(c u p) r -> p c u r",
                                   c=NSC, u=16)[:, c, :, :])
            for u in range(16):
                nc.tensor.matmul(
                    fps0[:, :], lhsT=ft_t[:, u, 0:M], rhs=mh_t[:, u, :],
                    start=(c == 0 and u == 0),
                    stop=(c == NSC - 1 and u == 15))
                nc.tensor.matmul(
                    fps1[:, :], lhsT=ft_t[:, u, M:D2], rhs=mh_t[:, u, :],
                    start=(c == 0 and u == 0),
                    stop=(c == NSC - 1 and u == 15))
        ot = pT.tile([M, 2, nbr], dt.float32, tag="ot")
        nc.vector.tensor_copy(ot[:, 0, :], fps0[:, :])
        nc.vector.tensor_copy(ot[:, 1, :], fps1[:, :])
        nc.sync.dma_start(
            fpart[:].rearrange("r (k o) -> o k r", k=2), ot[:])

    nc.compile()
    return nc


def _prep_B(inputs, branches, n_pad, sp, aligned):
    """branches: list of (X_sorted [S, D2] f32, n_valid, sign)."""
    nbr = len(branches)
    TT = n_pad // 128
    qkv_np = np.zeros((2, 3, H, 128, D2), np.float32)
    for w_i, wkey, bkey in ((0, "qw", "qb"), (1, "kw", "kb"), (2, "vw", "vb")):
        wa = np.asarray(inputs[wkey], np.float32)   # [H, 200, 200] (out, in)
        ba = np.asarray(inputs[bkey], np.float32)   # [H, 200]
        for h in range(H):
            wT = np.zeros((256, D2), np.float32)
            wT[:D2] = wa[h].T
            if w_i != 2:
                wT[D2] = ba[h]   # v bias added in f32 on device instead
            qkv_np[0, w_i, h] = wT[:128]
            qkv_np[1, w_i, h] = wT[128:]
    vbias_np = np.ascontiguousarray(np.broadcast_to(
        np.asarray(inputs["vb"], np.float32)[None, :, :], (128, H, D2)))
    qkv_sb = np.ascontiguousarray(
        qkv_np.transpose(3, 2, 0, 1, 4)).astype(bf)  # [128, H, 2, 3, 200]

    cw = np.asarray(inputs["concat_w"], np.float32)  # [200, 1600]
    cwp = np.zeros((2048, D2), np.float32)
    for h in range(H):
        cwp[256 * h:256 * h + D2] = cw[:, D2 * h:D2 * (h + 1)].T
    cw_sb = np.ascontiguousarray(cwp.reshape(16, 128, D2).transpose(1, 0, 2))
    cbias_np = np.ascontiguousarray(
        np.asarray(inputs["concat_b"], np.float32).reshape(2, M).T)

    xf_list = []
    vm = np.zeros((128, nbr, 128), np.float32)
    for b, (Xs, n, sign) in enumerate(branches):
        xfp = np.zeros((256, n_pad), np.float32)
        xfp[:D2, :n] = Xs[:n].T
        xfp[D2, :n] = 1.0
        xf_list.append(np.ascontiguousarray(
            xfp.reshape(2, 128, n_pad).transpose(1, 0, 2)).astype(bf))
        for tt in range(TT):
            valid = (np.arange(128) + 128 * tt) < n
            for h in range(H):
                vm[:, b, h * TT + tt] = valid

    featT = np.ascontiguousarray(np.asarray(inputs["feat_w"], np.float32).T)
    NROW = sp * D2
    NSC = (NROW + 2047) // 2048

    in_maps = []
    for c in range(NCORES):
        s0 = c * sp
        m = {"qkv": qkv_sb, "cwb": cw_sb.astype(bf), "cwf": cw_sb,
             "cbias": cbias_np, "vmask": vm, "vbias": vbias_np}
        sm = np.zeros((1, nbr, sp), np.float32)
        for b, (Xs, n, sign) in enumerate(branches):
            xq = np.zeros((256, sp), np.float32)
            valid_cols = max(0, min(sp, n - s0))
            if valid_cols > 0:
                xq[:D2, :valid_cols] = Xs[s0:s0 + valid_cols].T
                xq[D2, :valid_cols] = 1.0
                sm[0, b, :valid_cols] = 1.0
            m[f"xf{b}"] = xf_list[b]
            m[f"xq{b}"] = np.ascontiguousarray(
                xq.reshape(2, 128, sp).transpose(1, 0, 2)).astype(bf)
        m["smask"] = sm
        stripe = np.zeros((NSC * 2048, D2), np.float32)
        r0 = s0 * D2
        rows = max(0, min(NROW, featT.shape[0] - r0))
        if rows > 0:
            stripe[:rows] = featT[r0:r0 + rows]
        m["ftd"] = np.ascontiguousarray(
            stripe.reshape(NSC, 16, 128, D2).transpose(0, 2, 1, 3))
        in_maps.append(m)
    return in_maps


def _run_B(inputs, branches):
    from concourse.bass_utils import run_bass_kernel_spmd
    nmax = max(n for _, n, _ in branches)
    sp = -(-nmax // (NCORES * 128)) * 128
    n_pad = sp * NCORES
    aligned = all(n == n_pad for _, n, _ in branches)
    signs = tuple(sign for _, _, sign in branches)
    key = (n_pad, sp, len(branches), signs, aligned)
    if key not in _cacheB:
        _cacheB[key] = _build_B(key)
    nc = _cacheB[key]
    in_maps = _prep_B(inputs, branches, n_pad, sp, aligned)
    res = run_bass_kernel_spmd(nc, in_maps, list(range(NCORES)))
    parts = np.stack([res.results[c]["fpart"] for c in range(NCORES)])
    return parts.sum(axis=0)  # [nbr, 200]


# ------------------------------------------------------------------ driver

def kernel(**inputs):
    hid = _run_A(inputs)

    head, sents = hid[0], hid[1:]
    u = head.astype(np.float32) @ np.asarray(inputs["sim_w"], np.float32)
    logits = sents @ u + np.asarray(inputs["sim_b"], np.float32)[0]
    sig = (1.0 / (1.0 + np.exp(-logits))).astype(np.float32)
    e = np.exp(sig - sig.max())
    prob = (e / e.sum()).astype(np.float32)
    attend = (prob[:, None] * sents).astype(np.float32)

    mask = sig >= 0.5
    n_high = int(mask.sum())
    n_low = S - n_high
    feat_b = np.asarray(inputs["feat_b"], np.float32)

    branches = []
    slot = {}
    if n_high > 0:
        order = np.argsort(np.where(mask, 0, 1), kind="stable")
        slot[0] = len(branches)
        branches.append((attend[order], n_high, 1.0))
    if n_low > 0:
        order = np.argsort(np.where(mask, 1, 0), kind="stable")
        slot[1] = len(branches)
        branches.append((attend[order], n_low, -1.0))

    out = np.zeros((2, D2), np.float32)
    if branches:
        parts = _run_B(inputs, branches)
        for r in range(2):
            out[r] = (parts[slot[r]] + feat_b) if r in slot else feat_b
    else:
        out[0] = feat_b
        out[1] = feat_b
    return out.astype(np.float32)
